# revision 36
# baseline (speedup 1.0000x reference)
"""Trainium2 Bass kernel for nn_DiscriminatorModel (8-layer MLP with
LayerNorm+LeakyReLU, 524288x128 input, data-parallel over 8 NeuronCores).

Evacuation-balanced redesign of the previous single-word-fp16 kernel.
Profiling showed the old kernel was bound by PSUM-evacuation work on the
Scalar (ACT) engine (71% busy incl. DMA dispatch) with the PE array only
~45% loaded. Changes:

  - Math identical to before: mean-centering folded into weights host-side,
    per-row rsqrt deferred to the end via E8 = v7' + eps*k7^2*v6', per-layer
    power-of-2 scale calibration, low-E8 rows recomputed on host in float64.
  - L4 now also uses the relu-trick: DVE evacuates r4 = 0.8*relu(u4) (one
    tensor_scalar, keeping the clock-setting ACT queue one op lighter) and
    the 0.2*u4 linear part rides into L5 via composed stationaries
    C[h] = 0.2 * T4 @ T5[64h:64h+64, :] streaming a3 once more. This takes
    L4's Prelu off the critical ACT queue.
  - t6/t7 fp16 staging on DVE, squares on the (otherwise idle) Pool engine;
    ACT runs a single Prelu table the whole kernel (r4's 0.8*relu is
    Prelu(scale=0.8, alpha=0) - same table).
  - Division by sqrt(E8) moved to the host: the kernel ships raw per-row
    y and E8 ([128,128] psum block per supertile, one DMA each), dropping
    the on-device sqrt/reciprocal/multiply epilogue entirely.
  - All DMA dispatch on the Sync queue (the old kernel burned 13us of ACT
    issuing DMA descriptors).
  - L1 matmuls run on 4 PE column strips (measured 6.8 cols/ns vs 2.2
    serial); evacuation tiles are [128,1024] (ACT 114 Ge/s, DVE 104 Ge/s).

Requires all LayerNorm beta == 0 and gamma > 0 (true for the reference
inputs); otherwise falls back to a float64 numpy forward pass.
"""

import numpy as np

EPS = 1e-5
SLOPE = 0.2
DIMS = [128, 32, 64, 32, 16, 8, 4, 2]
N_CORES = 8
ROWS = 524288
RPC = ROWS // N_CORES        # 65536 rows per core
R_ST = 8192                  # rows per supertile
N_ST = RPC // R_ST           # 8 supertiles per core
F16 = np.float16
SIGMA_T = 16.0               # per-layer target std after scaling
FLAG_RATIO = 6e-2            # host-patch rows with E8 < ratio*median


def _center(W):
    d = W.shape[1]
    return np.asarray(W, np.float64) @ (np.eye(d) - 1.0 / d)


def _blockdiag(W, c):
    din, dout = W.shape
    out = np.zeros((c * din, c * dout), W.dtype)
    for b in range(c):
        out[b * din:(b + 1) * din, b * dout:(b + 1) * dout] = W
    return out


def _pos(c_out):
    """Output-block position map for split transitions: even blocks to the
    low partition half, odd blocks to the high half."""
    return lambda b: (b % 2) * (c_out // 2) + (b // 2)


def _transition_stat_split(W, c_in, pos_in):
    """Stationary for a c_in -> 2*c_in transition in parity-SPLIT layout."""
    din, dout = W.shape
    w = 128 // (2 * c_in)
    assert w == dout
    S = np.zeros((128, 64), np.float64)
    for g in range(c_in):
        rp = pos_in(g) * din
        S[rp:rp + din, g * w:(g + 1) * w] = W
    return S


def _var_stats(g6, g7, pos6, pos7):
    """vpk stationaries for v6 (par a/b) and v7, with 1/(d*g^2) weights."""
    d6, d7 = DIMS[6], DIMS[7]
    w6 = 1.0 / (d6 * np.square(np.asarray(g6, np.float64)))
    w7 = 1.0 / (d7 * np.square(np.asarray(g7, np.float64)))
    V6 = []
    for par in range(2):
        S = np.zeros((128, 64), np.float64)
        for m in range(64):
            if m % 2 != par:
                continue
            g = m // 2
            rp = pos6(g) * d6
            S[rp:rp + d6, m] = w6
        V6.append(S)
    V7 = np.zeros((128, 64), np.float64)
    for m in range(64):
        rp = pos7(m) * d7
        V7[rp:rp + d7, m] = w7
    return V6[0], V6[1], V7


def _ref_rows(inp, idx):
    """float64 reference forward for a subset of rows."""
    h = np.asarray(inp["x"], np.float32)[idx].astype(np.float64)
    for i in range(7):
        W = np.asarray(inp[f"W{i+1}"], np.float32).astype(np.float64)
        gg = np.asarray(inp[f"g{i+1}"], np.float32).astype(np.float64)
        bb = np.asarray(inp[f"bt{i+1}"], np.float32).astype(np.float64)
        h = h @ W
        m = h.mean(-1, keepdims=True)
        v = np.square(h - m).mean(-1, keepdims=True)
        h = (h - m) / np.sqrt(v + EPS) * gg + bb
        h = np.where(h > 0, h, SLOPE * h)
    return (h @ np.asarray(inp["W8"], np.float32).astype(np.float64)
            + np.asarray(inp["b8"], np.float32).astype(np.float64))


def _numpy_forward(inp):
    return _ref_rows(inp, slice(None)).astype(np.float32)


def _calibrate(inp, Sg):
    """Per-layer power-of-2 scales so std(u_l') ~= SIGMA_T."""
    xs = np.asarray(inp["x"], np.float32)[:4096].astype(np.float64)
    ks = []
    h, C = xs, 1.0
    for i in range(7):
        u = h @ Sg[i]
        s = float(u.std())
        if not np.isfinite(s) or s <= 0:
            k = 1.0
        else:
            k = float(2.0 ** np.round(np.log2(SIGMA_T / (C * s))))
        ks.append(k)
        C *= k
        h = np.where(u > 0, u, SLOPE * u)
    return ks


def _build_consts(inp):
    g = [np.asarray(inp[f"g{l}"], np.float32).astype(np.float64)
         for l in range(1, 8)]
    Sg = [_center(np.asarray(inp[f"W{l}"], np.float32)) @ np.diag(g[l - 1])
          for l in range(1, 8)]
    ks = _calibrate(inp, Sg)
    S = [Sg[i] * ks[i] for i in range(7)]
    W8 = np.asarray(inp["W8"], np.float32).astype(np.float64)

    cols = {}
    def add(name, arr):
        cols[name] = arr.astype(F16)

    add("s1", S[0])                                    # [128, 32]
    bd2 = _blockdiag(S[1], 2)                          # [64, 128]
    add("s2", np.vstack([bd2, bd2]))                   # [128, 128]
    s3 = _blockdiag(S[2], 2)                           # [128, 64]
    add("s3", s3)
    sc = 0.2 * (bd2 @ s3)                              # [64, 64]
    add("sc", np.vstack([sc, sc]))                     # [128, 64]
    t4 = _transition_stat_split(S[3], 4, lambda b: b)
    add("t4", t4)
    t5 = _transition_stat_split(S[4], 8, _pos(8))
    add("t5", t5)
    add("t6", _transition_stat_split(S[5], 16, _pos(16)))
    add("t7", _transition_stat_split(S[6], 32, _pos(32)))
    # composed stationaries for the L4 relu-trick (0.2*u4 into L5)
    add("c0", 0.2 * (t4 @ t5[0:64, :]))                # [128, 64]
    add("c1", 0.2 * (t4 @ t5[64:128, :]))              # [128, 64]
    pos6, pos7 = _pos(32), _pos(64)
    s8 = np.zeros((128, 64), np.float64)
    for m in range(64):
        rp = pos7(m) * 2
        s8[rp:rp + 2, m] = W8[:, 0]
    add("s8", s8)                                      # [128, 64]
    V6a, V6b, V7 = _var_stats(g[5], g[6], pos6, pos7)
    add("v6a", V6a)
    add("v6b", V6b)
    add("v7", V7)

    order = sorted(cols.keys())
    offs, total = {}, 0
    wcols = {}
    for k in order:
        offs[k] = total
        wcols[k] = cols[k].shape[1]
        total += cols[k].shape[1]
    wpack = np.zeros((128, total), F16)
    for k in order:
        wpack[:, offs[k]:offs[k] + cols[k].shape[1]] = cols[k]
    return wpack, offs, wcols, ks


def _split_multi_waits(nc):
    """Walrus build limit: <=1 sync wait per instruction. Hoist extras onto
    same-engine NOPs inserted just before the instruction."""
    import concourse.mybir as mybir
    import bass_rust
    cnt = 0
    for f in nc.m.functions:
        for blk in f.blocks:
            newlist = []
            for inst in blk.instructions:
                si = inst.sync_info
                waits = list(si.on_wait) if si is not None and si.on_wait else []
                if len(waits) > 1:
                    for w in waits[:-1]:
                        nop = mybir.InstNoOp(name=f"waitnop_{cnt}", ins=[], outs=[])
                        cnt += 1
                        nop.engine = inst.engine
                        nop.sync_info = bass_rust.SyncInfo(on_wait=[w], on_update=[])
                        newlist.append(nop)
                    inst.sync_info = bass_rust.SyncInfo(
                        on_wait=[waits[-1]], on_update=list(si.on_update))
                newlist.append(inst)
            blk.instructions = newlist
    return cnt


_WCOLS = {}


def _build_program(offs, wpack_cols, sq6_scale):
    import concourse.bass as bass
    import concourse.mybir as mybir
    from concourse.tile import TileContext
    from contextlib import ExitStack

    import bass_rust
    from concourse.tile import TileContext as _TC
    from concourse.vector_clock import ScopedClock

    def _patched_drain(self, tick_clock, wait_clock):
        probe = self.nc.sync.nop()
        wait_clock.add_sem_waits(probe.ins,
                                 ScopedClock({None: tick_clock.global_clock}))
        si = probe.ins.sync_info
        waits = list(si.on_wait) if si is not None else []
        upd = list(si.on_update) if si is not None else []
        probe.ins.sync_info = bass_rust.SyncInfo(on_wait=waits[:1], on_update=upd)
        for w in waits[1:]:
            nop = self.nc.sync.nop()
            nop.ins.sync_info = bass_rust.SyncInfo(on_wait=[w], on_update=[])
        self.nc.sync.drain()
        self.nc.all_engine_barrier()
        assert self.sems is not None
        popped = self.nc._tile_sem_poison_stack.pop()
        assert popped is self._sem_poison
        self.nc.clear_and_free_semaphores(list(self.sems.allocated().values()))
        self.nc.all_engine_barrier()

    _TC._drain_and_barrier = _patched_drain

    f16, f32 = mybir.dt.float16, mybir.dt.float32
    AF = mybir.ActivationFunctionType
    OP = mybir.AluOpType

    nc = bass.Bass(trn_type="TRN2", num_swdge_queues=4)
    xhi_d = nc.dram_tensor("xhi", [128, RPC], f16, kind="ExternalInput")
    wp_d = nc.dram_tensor("wpack", [128, wpack_cols], f16, kind="ExternalInput")
    ey_d = nc.dram_tensor("ey", [128, N_ST * 128], f32, kind="ExternalOutput")

    with TileContext(nc) as tc:
        with ExitStack() as ctx:
            const = ctx.enter_context(tc.tile_pool(name="const", bufs=1))
            # dummy activation with no deps: pulls the ACT table load into
            # the preamble shadow
            warmup = const.tile([128, 1], f32, name="warmup")
            nc.scalar.activation(warmup[:, :], warmup[:, :], AF.Prelu,
                                 bias=0.0, scale=1.0, alpha=SLOPE)
            wp = const.tile([128, wpack_cols], f16)
            nc.sync.dma_start(wp[:, :], wp_d[:, :])

            def W(name):
                return wp[:, offs[name]:offs[name] + _WCOLS[name]]

            xp = ctx.enter_context(tc.tile_pool(name="xp", bufs=12))
            ap = ctx.enter_context(tc.tile_pool(name="ap", bufs=3))
            fin = ctx.enter_context(tc.tile_pool(name="fin", bufs=1))
            pu = ctx.enter_context(tc.tile_pool(name="pu", bufs=2, space="PSUM"))
            pv = ctx.enter_context(tc.tile_pool(name="pv", bufs=2, space="PSUM"))

            outsb = fin.tile([128, N_ST * 128], f32, name="outsb", tag="outsb")

            s2q = [wp[:, offs["s2"]:offs["s2"] + 128][64 * q:64 * (q + 1), :]
                   for q in range(2)]
            scq = [wp[:, offs["sc"]:offs["sc"] + 64][64 * q:64 * (q + 1), :]
                   for q in range(2)]

            state = {}

            def emit_dma(st):
                x0 = st * R_ST
                xch = []
                for kk in range(4):
                    t = xp.tile([128, 2048], f16, name=f"xc{kk}", tag="xc")
                    nc.sync.dma_start(
                        t[:, :], xhi_d[:, x0 + 2048 * kk:x0 + 2048 * (kk + 1)])
                    xch.append(t)
                state[st] = {"xch": xch}

            def emit_l1_chunk(st, c):
                xch = state[st]["xch"]
                if c == 0:
                    state[st]["a1"] = ap.tile([128, 2048], f16, name="a1",
                                              tag="a1")
                a1 = state[st]["a1"]
                u = pu.tile([128, 1024], f32, name="u", tag="u")
                if st == 0:
                    # ramp: consume DMA chunks in arrival order (b-outer)
                    for b in range(4):
                        for o in (0, 512):
                            rhs = xch[b][:, 1024 * c + o:1024 * c + o + 512]
                            nc.tensor.matmul(
                                u[32 * b:32 * (b + 1), o:o + 512], W("s1"), rhs,
                                start=True, stop=True, tile_position=(0, 32 * b))
                else:
                    for o in (0, 512):
                        for b in range(4):
                            rhs = xch[b][:, 1024 * c + o:1024 * c + o + 512]
                            nc.tensor.matmul(
                                u[32 * b:32 * (b + 1), o:o + 512], W("s1"), rhs,
                                start=True, stop=True, tile_position=(0, 32 * b))
                nc.scalar.activation(a1[:, 1024 * c:1024 * (c + 1)], u[:, :],
                                     AF.Prelu, bias=0.0, scale=1.0,
                                     alpha=SLOPE)

            def emit_l2_chunk(st, c):
                a1 = state[st]["a1"]
                if c == 0:
                    state[st]["r2"] = [
                        ap.tile([128, 2048], f16, name=f"r2{q}", tag=f"r2{q}")
                        for q in range(2)]
                r2 = state[st]["r2"]
                for q in range(2):
                    us = pv.tile([128, 1024], f32, name="uv2", tag="v")
                    for o in (0, 512):
                        rhs = a1[64 * q:64 * (q + 1),
                                 1024 * c + o:1024 * c + o + 512]
                        nc.tensor.matmul(
                            us[:, o:o + 512], s2q[q], rhs,
                            start=True, stop=True, tile_position=(64 * q, 0))
                    if c == 0 and q == 0:
                        # queue balance: ACT (Prelu scale=0.8 alpha=0 = the
                        # same relu-trick, same table) takes half this tile
                        nc.scalar.activation(
                            r2[q][:, 0:512], us[:, 0:512], AF.Prelu,
                            bias=0.0, scale=0.8, alpha=0.0)
                        nc.vector.tensor_scalar(
                            r2[q][:, 512:1024], us[:, 512:1024],
                            0.8, 0.0, OP.mult, OP.max)
                    else:
                        nc.vector.tensor_scalar(
                            r2[q][:, 1024 * c:1024 * (c + 1)], us[:, :],
                            0.8, 0.0, OP.mult, OP.max)

            def emit_l3_chunk(st, c):
                a1, r2 = state[st]["a1"], state[st]["r2"]
                if c == 0:
                    state[st]["a3"] = ap.tile([128, 2048], f16, name="a3",
                                              tag="a3")
                a3 = state[st]["a3"]
                u = pv.tile([128, 1024], f32, name="u3", tag="v")
                # all s3 matmuls first, then all composed ones: adjacent
                # queue entries target independent regions so the PE can
                # overlap them (s3->sc accumulation pairs would serialize)
                for o in (0, 512):
                    for q in range(2):
                        nc.tensor.matmul(
                            u[64 * q:64 * (q + 1), o:o + 512], W("s3"),
                            r2[q][:, 1024 * c + o:1024 * c + o + 512],
                            start=True, stop=False, tile_position=(0, 64 * q))
                for o in (0, 512):
                    for q in range(2):
                        nc.tensor.matmul(
                            u[64 * q:64 * (q + 1), o:o + 512], scq[q],
                            a1[64 * q:64 * (q + 1),
                               1024 * c + o:1024 * c + o + 512],
                            start=False, stop=True,
                            tile_position=(64 * q, 64 * q))
                nc.scalar.activation(a3[:, 1024 * c:1024 * (c + 1)], u[:, :],
                                     AF.Prelu, bias=0.0, scale=1.0,
                                     alpha=SLOPE)

            def emit_t4(st):
                a3 = state[st]["a3"]
                r4 = ap.tile([128, 1024], f16, name="r4", tag="r4")
                state[st]["r4"] = r4
                u = pu.tile([128, 1024], f32, name="u4", tag="u")
                for par in (0, 1):
                    for o in (0, 512):
                        nc.tensor.matmul(
                            u[64 * par:64 * par + 64, o:o + 512], W("t4"),
                            a3[:, 1024 * par + o:1024 * par + o + 512],
                            start=True, stop=True, tile_position=(0, 64 * par))
                # r4 = 0.8*relu(u4), all on DVE: ACT is the step clock
                nc.vector.tensor_scalar(r4[:, :], u[:, :],
                                        0.8, 0.0, OP.mult, OP.max)

            def emit_t5(st):
                a3, r4 = state[st]["a3"], state[st]["r4"]
                a5 = ap.tile([128, 512], f16, name="a5", tag="a5")
                state[st]["a5"] = a5
                u = pu.tile([128, 1024], f32, name="u5", tag="u")
                uv = u[:, 0:512]
                # group by stationary (not by parity): parity pairs are on
                # independent strips and can overlap in the array
                for par in (0, 1):
                    nc.tensor.matmul(uv[64 * par:64 * par + 64, :], W("t5"),
                                     r4[:, 512 * par:512 * par + 512],
                                     start=True, stop=False,
                                     tile_position=(0, 64 * par))
                for par in (0, 1):
                    nc.tensor.matmul(
                        uv[64 * par:64 * par + 64, :], W("c0"),
                        a3[:, 512 * par:512 * par + 512],
                        start=False, stop=False, tile_position=(0, 64 * par))
                for par in (0, 1):
                    nc.tensor.matmul(
                        uv[64 * par:64 * par + 64, :], W("c1"),
                        a3[:, 1024 + 512 * par:1024 + 512 * par + 512],
                        start=False, stop=True, tile_position=(0, 64 * par))
                nc.scalar.activation(a5[:, :], uv, AF.Prelu, bias=0.0,
                                     scale=1.0, alpha=SLOPE)

            def emit_t6(st):
                """t6 into a shared [128,1024] psum tile: u6 at [0:256],
                u7 at [256:384], e8/y at [384:512]."""
                a5 = state[st]["a5"]
                u = pu.tile([128, 1024], f32, name="uttt", tag="u")
                state[st]["uttt"] = u
                u6 = u[:, 0:256]
                a6 = ap.tile([128, 256], f16, name="a6", tag="a6")
                t6c = ap.tile([128, 256], f16, name="t6c", tag="t6c")
                s6 = ap.tile([128, 256], f16, name="s6", tag="s6")
                state[st]["a6"], state[st]["s6"] = a6, s6
                for par in (0, 1):
                    nc.tensor.matmul(u6[64 * par:64 * par + 64, :], W("t6"),
                                     a5[:, 256 * par:256 * par + 256],
                                     start=True, stop=True,
                                     tile_position=(0, 64 * par))
                nc.scalar.activation(a6[:, :], u6, AF.Prelu, bias=0.0,
                                     scale=1.0, alpha=SLOPE)
                nc.vector.tensor_scalar(t6c[:, :], u6, sq6_scale, None,
                                        OP.mult)
                nc.gpsimd.tensor_tensor(s6[:, :], t6c[:, :], t6c[:, :],
                                        OP.mult)

            def emit_t7(st):
                u, a6 = state[st]["uttt"], state[st]["a6"]
                u7 = u[:, 256:384]
                a7 = ap.tile([128, 128], f16, name="a7", tag="a7")
                t7c = ap.tile([128, 128], f16, name="t7c", tag="t7c")
                s7 = ap.tile([128, 128], f16, name="s7", tag="s7")
                state[st]["a7"], state[st]["s7"] = a7, s7
                for par in (0, 1):
                    nc.tensor.matmul(u7[64 * par:64 * par + 64, :], W("t7"),
                                     a6[:, 128 * par:128 * par + 128],
                                     start=True, stop=True,
                                     tile_position=(0, 64 * par))
                nc.scalar.activation(a7[:, :], u7, AF.Prelu, bias=0.0,
                                     scale=1.0, alpha=SLOPE)
                nc.vector.tensor_copy(t7c[:, :], u7)
                nc.gpsimd.tensor_tensor(s7[:, :], t7c[:, :], t7c[:, :],
                                        OP.mult)

            def emit_tails(st):
                u = state[st]["uttt"]
                a7, s6, s7 = state[st]["a7"], state[st]["s6"], state[st]["s7"]
                ey = u[:, 384:512]
                # tails: e8 at partitions 0:64, y at 64:128 of ey
                nc.tensor.matmul(ey[0:64, :], W("v7"), s7[:, :],
                                 start=True, stop=False)
                nc.tensor.matmul(ey[0:64, :], W("v6a"), s6[:, 0:128],
                                 start=False, stop=False)
                nc.tensor.matmul(ey[0:64, :], W("v6b"), s6[:, 128:256],
                                 start=False, stop=True)
                nc.tensor.matmul(ey[64:128, :], W("s8"), a7[:, :],
                                 start=True, stop=True, tile_position=(0, 64))
                nc.vector.tensor_copy(
                    outsb[:, st * 128:(st + 1) * 128], u[:, 384:512])
                nc.sync.dma_start(ey_d[:, st * 128:(st + 1) * 128],
                                  outsb[:, st * 128:(st + 1) * 128])
                state[st].clear()

            # Wavefront pipeline across supertiles: stage offsets
            # L1(t), L2(t-1), L3(t-2), t4(t-3), t5(t-4), ttt(t-5).
            # t5 one step after t4 so t5's matmuls never wait on the r4
            # evacuation within the same step (ditto ttt after t5).
            NSTG = 6
            emit_dma(0)
            emit_dma(1)
            for t in range(N_ST + NSTG - 1):
                if t + 2 < N_ST:
                    emit_dma(t + 2)

                def valid(off):
                    return 0 <= t - off < N_ST

                # piece order: dep-chained pieces (t6 -> t7 -> tails feed off
                # same-step ACT/Pool results) are spaced out between ready
                # streams so the PE's 4-deep dependency wait-queue never
                # head-of-line-blocks ready work
                pieces = []
                if valid(1):
                    pieces.append(lambda: emit_l2_chunk(t - 1, 0))
                if valid(0):
                    pieces.append(lambda: emit_l1_chunk(t, 0))
                if valid(3):
                    pieces.append(lambda: emit_t4(t - 3))
                if valid(2):
                    pieces.append(lambda: emit_l3_chunk(t - 2, 0))
                if valid(4):
                    pieces.append(lambda: emit_t5(t - 4))
                if valid(0):
                    pieces.append(lambda: emit_l1_chunk(t, 1))
                if valid(5):
                    pieces.append(lambda: emit_t6(t - 5))
                if valid(1):
                    pieces.append(lambda: emit_l2_chunk(t - 1, 1))
                if valid(5):
                    pieces.append(lambda: emit_t7(t - 5))
                if valid(2):
                    pieces.append(lambda: emit_l3_chunk(t - 2, 1))
                if valid(5):
                    pieces.append(lambda: emit_tails(t - 5))
                for p in pieces:
                    p()
    _split_multi_waits(nc)
    return nc


def kernel(**inputs):
    for l in range(1, 8):
        if np.abs(np.asarray(inputs[f"bt{l}"], np.float32)).max() > 0:
            return _numpy_forward(inputs)
        if np.asarray(inputs[f"g{l}"], np.float32).min() <= 0:
            return _numpy_forward(inputs)

    wpack, offs, wcols, ks = _build_consts(inputs)
    global _WCOLS
    _WCOLS = wcols

    x = np.asarray(inputs["x"], np.float32)
    xT = np.ascontiguousarray(x.T)               # [128, 524288]
    xhi = xT.astype(F16)
    b8 = float(np.asarray(inputs["b8"], np.float32).reshape(-1)[0])
    sq6_scale = float(np.sqrt(EPS) * ks[6])      # (scale*u6)^2 = eps*k7^2*u6^2

    nc = _build_program(offs, wpack.shape[1], sq6_scale)

    in_maps = []
    for c in range(N_CORES):
        s = slice(c * RPC, (c + 1) * RPC)
        in_maps.append({
            "xhi": np.ascontiguousarray(xhi[:, s]),
            "wpack": wpack,
        })

    from concourse.bass_utils import run_bass_kernel_spmd
    res = run_bass_kernel_spmd(nc, in_maps, core_ids=list(range(N_CORES)))

    # device ey layout [128, N_ST*128]: per supertile block [128,128]:
    # partitions 0:64 = E8, 64:128 = y; row-in-core = st*8192 + m*128 + j
    def unpack(arr):
        a = arr.reshape(128, N_ST, 128)
        e8 = a[0:64].transpose(1, 0, 2).reshape(-1)
        y = a[64:128].transpose(1, 0, 2).reshape(-1)
        return e8, y

    out = np.empty((ROWS, 1), np.float32)
    e8r = np.empty(ROWS, np.float64)
    for c in range(N_CORES):
        e8, y = unpack(res.results[c]["ey"].astype(np.float64))
        with np.errstate(invalid="ignore", divide="ignore"):
            out[c * RPC:(c + 1) * RPC, 0] = (y / np.sqrt(e8) + b8)
        e8r[c * RPC:(c + 1) * RPC] = e8

    finite = np.isfinite(e8r) & (e8r > 0)
    med = float(np.median(e8r[finite])) if finite.any() else 0.0
    if not np.isfinite(med) or med <= 0:
        return _numpy_forward(inputs).reshape(ROWS, 1)
    bad = (~finite) | (e8r < FLAG_RATIO * med) | (~np.isfinite(out[:, 0]))
    if bad.mean() > 0.30:
        return _numpy_forward(inputs).reshape(ROWS, 1)
    idx = np.nonzero(bad)[0]
    if idx.size:
        out[idx, 0] = _ref_rows(inputs, idx).astype(np.float32).ravel()
    return out


# revision 37
# speedup vs baseline: 1.0062x; 1.0062x over previous
"""Trainium2 Bass kernel for nn_DiscriminatorModel (8-layer MLP with
LayerNorm+LeakyReLU, 524288x128 input, data-parallel over 8 NeuronCores).

Evacuation-balanced redesign of the previous single-word-fp16 kernel.
Profiling showed the old kernel was bound by PSUM-evacuation work on the
Scalar (ACT) engine (71% busy incl. DMA dispatch) with the PE array only
~45% loaded. Changes:

  - Math identical to before: mean-centering folded into weights host-side,
    per-row rsqrt deferred to the end via E8 = v7' + eps*k7^2*v6', per-layer
    power-of-2 scale calibration, low-E8 rows recomputed on host in float64.
  - L4 now also uses the relu-trick: DVE/ACT evacuate r4 = 0.8*relu(u4) (a
    one-op tensor_scalar / Prelu(alpha=0) split between both engines) and
    the 0.2*u4 linear part rides into L5 via composed stationaries
    C[h] = 0.2 * T4 @ T5[64h:64h+64, :] streaming a3 once more. This takes
    L4's Prelu off the critical ACT queue.
  - t6/t7 fp16 staging on DVE, squares on the (otherwise idle) Pool engine;
    ACT runs a single Prelu table the whole kernel (r4's 0.8*relu is
    Prelu(scale=0.8, alpha=0) - same table).
  - Division by sqrt(E8) moved to the host: the kernel ships raw per-row
    y and E8 ([128,128] psum block per supertile, one DMA each), dropping
    the on-device sqrt/reciprocal/multiply epilogue entirely.
  - All DMA dispatch on the Sync queue (the old kernel burned 13us of ACT
    issuing DMA descriptors).
  - L1 matmuls run on 4 PE column strips (measured 6.8 cols/ns vs 2.2
    serial); evacuation tiles are [128,1024] (ACT 114 Ge/s, DVE 104 Ge/s).

Requires all LayerNorm beta == 0 and gamma > 0 (true for the reference
inputs); otherwise falls back to a float64 numpy forward pass.
"""

import numpy as np

EPS = 1e-5
SLOPE = 0.2
DIMS = [128, 32, 64, 32, 16, 8, 4, 2]
N_CORES = 8
ROWS = 524288
RPC = ROWS // N_CORES        # 65536 rows per core
R_ST = 8192                  # rows per supertile
N_ST = RPC // R_ST           # 8 supertiles per core
F16 = np.float16
SIGMA_T = 16.0               # per-layer target std after scaling
FLAG_RATIO = 6e-2            # host-patch rows with E8 < ratio*median


def _center(W):
    d = W.shape[1]
    return np.asarray(W, np.float64) @ (np.eye(d) - 1.0 / d)


def _blockdiag(W, c):
    din, dout = W.shape
    out = np.zeros((c * din, c * dout), W.dtype)
    for b in range(c):
        out[b * din:(b + 1) * din, b * dout:(b + 1) * dout] = W
    return out


def _pos(c_out):
    """Output-block position map for split transitions: even blocks to the
    low partition half, odd blocks to the high half."""
    return lambda b: (b % 2) * (c_out // 2) + (b // 2)


def _transition_stat_split(W, c_in, pos_in):
    """Stationary for a c_in -> 2*c_in transition in parity-SPLIT layout."""
    din, dout = W.shape
    w = 128 // (2 * c_in)
    assert w == dout
    S = np.zeros((128, 64), np.float64)
    for g in range(c_in):
        rp = pos_in(g) * din
        S[rp:rp + din, g * w:(g + 1) * w] = W
    return S


def _var_stats(g6, g7, pos6, pos7):
    """vpk stationaries for v6 (par a/b) and v7, with 1/(d*g^2) weights."""
    d6, d7 = DIMS[6], DIMS[7]
    w6 = 1.0 / (d6 * np.square(np.asarray(g6, np.float64)))
    w7 = 1.0 / (d7 * np.square(np.asarray(g7, np.float64)))
    V6 = []
    for par in range(2):
        S = np.zeros((128, 64), np.float64)
        for m in range(64):
            if m % 2 != par:
                continue
            g = m // 2
            rp = pos6(g) * d6
            S[rp:rp + d6, m] = w6
        V6.append(S)
    V7 = np.zeros((128, 64), np.float64)
    for m in range(64):
        rp = pos7(m) * d7
        V7[rp:rp + d7, m] = w7
    return V6[0], V6[1], V7


def _ref_rows(inp, idx):
    """float64 reference forward for a subset of rows."""
    h = np.asarray(inp["x"], np.float32)[idx].astype(np.float64)
    for i in range(7):
        W = np.asarray(inp[f"W{i+1}"], np.float32).astype(np.float64)
        gg = np.asarray(inp[f"g{i+1}"], np.float32).astype(np.float64)
        bb = np.asarray(inp[f"bt{i+1}"], np.float32).astype(np.float64)
        h = h @ W
        m = h.mean(-1, keepdims=True)
        v = np.square(h - m).mean(-1, keepdims=True)
        h = (h - m) / np.sqrt(v + EPS) * gg + bb
        h = np.where(h > 0, h, SLOPE * h)
    return (h @ np.asarray(inp["W8"], np.float32).astype(np.float64)
            + np.asarray(inp["b8"], np.float32).astype(np.float64))


def _numpy_forward(inp):
    return _ref_rows(inp, slice(None)).astype(np.float32)


def _calibrate(inp, Sg):
    """Per-layer power-of-2 scales so std(u_l') ~= SIGMA_T."""
    xs = np.asarray(inp["x"], np.float32)[:4096].astype(np.float64)
    ks = []
    h, C = xs, 1.0
    for i in range(7):
        u = h @ Sg[i]
        s = float(u.std())
        if not np.isfinite(s) or s <= 0:
            k = 1.0
        else:
            k = float(2.0 ** np.round(np.log2(SIGMA_T / (C * s))))
        ks.append(k)
        C *= k
        h = np.where(u > 0, u, SLOPE * u)
    return ks


def _build_consts(inp):
    g = [np.asarray(inp[f"g{l}"], np.float32).astype(np.float64)
         for l in range(1, 8)]
    Sg = [_center(np.asarray(inp[f"W{l}"], np.float32)) @ np.diag(g[l - 1])
          for l in range(1, 8)]
    ks = _calibrate(inp, Sg)
    S = [Sg[i] * ks[i] for i in range(7)]
    W8 = np.asarray(inp["W8"], np.float32).astype(np.float64)

    cols = {}
    def add(name, arr):
        cols[name] = arr.astype(F16)

    add("s1", S[0])                                    # [128, 32]
    bd2 = _blockdiag(S[1], 2)                          # [64, 128]
    add("s2", np.vstack([bd2, bd2]))                   # [128, 128]
    s3 = _blockdiag(S[2], 2)                           # [128, 64]
    add("s3", s3)
    sc = 0.2 * (bd2 @ s3)                              # [64, 64]
    add("sc", np.vstack([sc, sc]))                     # [128, 64]
    t4 = _transition_stat_split(S[3], 4, lambda b: b)
    add("t4", t4)
    t5 = _transition_stat_split(S[4], 8, _pos(8))
    add("t5", t5)
    add("t6", _transition_stat_split(S[5], 16, _pos(16)))
    add("t7", _transition_stat_split(S[6], 32, _pos(32)))
    # composed stationaries for the L4 relu-trick (0.2*u4 into L5)
    add("c0", 0.2 * (t4 @ t5[0:64, :]))                # [128, 64]
    add("c1", 0.2 * (t4 @ t5[64:128, :]))              # [128, 64]
    pos6, pos7 = _pos(32), _pos(64)
    s8 = np.zeros((128, 64), np.float64)
    for m in range(64):
        rp = pos7(m) * 2
        s8[rp:rp + 2, m] = W8[:, 0]
    add("s8", s8)                                      # [128, 64]
    V6a, V6b, V7 = _var_stats(g[5], g[6], pos6, pos7)
    add("v6a", V6a)
    add("v6b", V6b)
    add("v7", V7)

    order = sorted(cols.keys())
    offs, total = {}, 0
    wcols = {}
    for k in order:
        offs[k] = total
        wcols[k] = cols[k].shape[1]
        total += cols[k].shape[1]
    wpack = np.zeros((128, total), F16)
    for k in order:
        wpack[:, offs[k]:offs[k] + cols[k].shape[1]] = cols[k]
    return wpack, offs, wcols, ks


def _split_multi_waits(nc):
    """Walrus build limit: <=1 sync wait per instruction. Hoist extras onto
    same-engine NOPs inserted just before the instruction."""
    import concourse.mybir as mybir
    import bass_rust
    cnt = 0
    for f in nc.m.functions:
        for blk in f.blocks:
            newlist = []
            for inst in blk.instructions:
                si = inst.sync_info
                waits = list(si.on_wait) if si is not None and si.on_wait else []
                if len(waits) > 1:
                    for w in waits[:-1]:
                        nop = mybir.InstNoOp(name=f"waitnop_{cnt}", ins=[], outs=[])
                        cnt += 1
                        nop.engine = inst.engine
                        nop.sync_info = bass_rust.SyncInfo(on_wait=[w], on_update=[])
                        newlist.append(nop)
                    inst.sync_info = bass_rust.SyncInfo(
                        on_wait=[waits[-1]], on_update=list(si.on_update))
                newlist.append(inst)
            blk.instructions = newlist
    return cnt


_WCOLS = {}


def _build_program(offs, wpack_cols, sq6_scale):
    import concourse.bass as bass
    import concourse.mybir as mybir
    from concourse.tile import TileContext
    from contextlib import ExitStack

    import bass_rust
    from concourse.tile import TileContext as _TC
    from concourse.vector_clock import ScopedClock

    def _patched_drain(self, tick_clock, wait_clock):
        probe = self.nc.sync.nop()
        wait_clock.add_sem_waits(probe.ins,
                                 ScopedClock({None: tick_clock.global_clock}))
        si = probe.ins.sync_info
        waits = list(si.on_wait) if si is not None else []
        upd = list(si.on_update) if si is not None else []
        probe.ins.sync_info = bass_rust.SyncInfo(on_wait=waits[:1], on_update=upd)
        for w in waits[1:]:
            nop = self.nc.sync.nop()
            nop.ins.sync_info = bass_rust.SyncInfo(on_wait=[w], on_update=[])
        self.nc.sync.drain()
        self.nc.all_engine_barrier()
        assert self.sems is not None
        popped = self.nc._tile_sem_poison_stack.pop()
        assert popped is self._sem_poison
        self.nc.clear_and_free_semaphores(list(self.sems.allocated().values()))
        self.nc.all_engine_barrier()

    _TC._drain_and_barrier = _patched_drain

    f16, f32 = mybir.dt.float16, mybir.dt.float32
    AF = mybir.ActivationFunctionType
    OP = mybir.AluOpType

    nc = bass.Bass(trn_type="TRN2", num_swdge_queues=4)
    xhi_d = nc.dram_tensor("xhi", [128, RPC], f16, kind="ExternalInput")
    wp_d = nc.dram_tensor("wpack", [128, wpack_cols], f16, kind="ExternalInput")
    ey_d = nc.dram_tensor("ey", [128, N_ST * 128], f32, kind="ExternalOutput")

    with TileContext(nc) as tc:
        with ExitStack() as ctx:
            const = ctx.enter_context(tc.tile_pool(name="const", bufs=1))
            # dummy activation with no deps: pulls the ACT table load into
            # the preamble shadow
            warmup = const.tile([128, 1], f32, name="warmup")
            nc.scalar.activation(warmup[:, :], warmup[:, :], AF.Prelu,
                                 bias=0.0, scale=1.0, alpha=SLOPE)
            wp = const.tile([128, wpack_cols], f16)
            nc.sync.dma_start(wp[:, :], wp_d[:, :])

            def W(name):
                return wp[:, offs[name]:offs[name] + _WCOLS[name]]

            xp = ctx.enter_context(tc.tile_pool(name="xp", bufs=12))
            ap = ctx.enter_context(tc.tile_pool(name="ap", bufs=3))
            fin = ctx.enter_context(tc.tile_pool(name="fin", bufs=1))
            pu = ctx.enter_context(tc.tile_pool(name="pu", bufs=2, space="PSUM"))
            pv = ctx.enter_context(tc.tile_pool(name="pv", bufs=2, space="PSUM"))

            outsb = fin.tile([128, N_ST * 128], f32, name="outsb", tag="outsb")

            s2q = [wp[:, offs["s2"]:offs["s2"] + 128][64 * q:64 * (q + 1), :]
                   for q in range(2)]
            scq = [wp[:, offs["sc"]:offs["sc"] + 64][64 * q:64 * (q + 1), :]
                   for q in range(2)]

            state = {}

            def emit_dma(st):
                x0 = st * R_ST
                xch = []
                for kk in range(4):
                    t = xp.tile([128, 2048], f16, name=f"xc{kk}", tag="xc")
                    nc.sync.dma_start(
                        t[:, :], xhi_d[:, x0 + 2048 * kk:x0 + 2048 * (kk + 1)])
                    xch.append(t)
                state[st] = {"xch": xch}

            def emit_l1_chunk(st, c):
                xch = state[st]["xch"]
                if c == 0:
                    state[st]["a1"] = ap.tile([128, 2048], f16, name="a1",
                                              tag="a1")
                a1 = state[st]["a1"]
                u = pu.tile([128, 1024], f32, name="u", tag="u")
                if st == 0:
                    # ramp: consume DMA chunks in arrival order (b-outer)
                    for b in range(4):
                        for o in (0, 512):
                            rhs = xch[b][:, 1024 * c + o:1024 * c + o + 512]
                            nc.tensor.matmul(
                                u[32 * b:32 * (b + 1), o:o + 512], W("s1"), rhs,
                                start=True, stop=True, tile_position=(0, 32 * b))
                else:
                    for o in (0, 512):
                        for b in range(4):
                            rhs = xch[b][:, 1024 * c + o:1024 * c + o + 512]
                            nc.tensor.matmul(
                                u[32 * b:32 * (b + 1), o:o + 512], W("s1"), rhs,
                                start=True, stop=True, tile_position=(0, 32 * b))
                nc.scalar.activation(a1[:, 1024 * c:1024 * (c + 1)], u[:, :],
                                     AF.Prelu, bias=0.0, scale=1.0,
                                     alpha=SLOPE)

            def emit_l2_chunk(st, c):
                a1 = state[st]["a1"]
                if c == 0:
                    state[st]["r2"] = [
                        ap.tile([128, 2048], f16, name=f"r2{q}", tag=f"r2{q}")
                        for q in range(2)]
                r2 = state[st]["r2"]
                for q in range(2):
                    us = pv.tile([128, 1024], f32, name="uv2", tag="v")
                    for o in (0, 512):
                        rhs = a1[64 * q:64 * (q + 1),
                                 1024 * c + o:1024 * c + o + 512]
                        nc.tensor.matmul(
                            us[:, o:o + 512], s2q[q], rhs,
                            start=True, stop=True, tile_position=(64 * q, 0))
                    nc.vector.tensor_scalar(
                        r2[q][:, 1024 * c:1024 * (c + 1)], us[:, :],
                        0.8, 0.0, OP.mult, OP.max)

            def emit_l3_chunk(st, c):
                a1, r2 = state[st]["a1"], state[st]["r2"]
                if c == 0:
                    state[st]["a3"] = ap.tile([128, 2048], f16, name="a3",
                                              tag="a3")
                a3 = state[st]["a3"]
                u = pv.tile([128, 1024], f32, name="u3", tag="v")
                # all s3 matmuls first, then all composed ones: adjacent
                # queue entries target independent regions so the PE can
                # overlap them (s3->sc accumulation pairs would serialize)
                for o in (0, 512):
                    for q in range(2):
                        nc.tensor.matmul(
                            u[64 * q:64 * (q + 1), o:o + 512], W("s3"),
                            r2[q][:, 1024 * c + o:1024 * c + o + 512],
                            start=True, stop=False, tile_position=(0, 64 * q))
                for o in (0, 512):
                    for q in range(2):
                        nc.tensor.matmul(
                            u[64 * q:64 * (q + 1), o:o + 512], scq[q],
                            a1[64 * q:64 * (q + 1),
                               1024 * c + o:1024 * c + o + 512],
                            start=False, stop=True,
                            tile_position=(64 * q, 64 * q))
                nc.scalar.activation(a3[:, 1024 * c:1024 * (c + 1)], u[:, :],
                                     AF.Prelu, bias=0.0, scale=1.0,
                                     alpha=SLOPE)

            def emit_t4(st):
                a3 = state[st]["a3"]
                r4 = ap.tile([128, 1024], f16, name="r4", tag="r4")
                state[st]["r4"] = r4
                u = pu.tile([128, 1024], f32, name="u4", tag="u")
                for par in (0, 1):
                    for o in (0, 512):
                        nc.tensor.matmul(
                            u[64 * par:64 * par + 64, o:o + 512], W("t4"),
                            a3[:, 1024 * par + o:1024 * par + o + 512],
                            start=True, stop=True, tile_position=(0, 64 * par))
                # r4 = 0.8*relu(u4), all on DVE: ACT is the step clock
                nc.vector.tensor_scalar(r4[:, :], u[:, :],
                                        0.8, 0.0, OP.mult, OP.max)

            def emit_t5(st):
                a3, r4 = state[st]["a3"], state[st]["r4"]
                a5 = ap.tile([128, 512], f16, name="a5", tag="a5")
                state[st]["a5"] = a5
                u = pu.tile([128, 1024], f32, name="u5", tag="u")
                uv = u[:, 0:512]
                # group by stationary (not by parity): parity pairs are on
                # independent strips and can overlap in the array
                for par in (0, 1):
                    nc.tensor.matmul(uv[64 * par:64 * par + 64, :], W("t5"),
                                     r4[:, 512 * par:512 * par + 512],
                                     start=True, stop=False,
                                     tile_position=(0, 64 * par))
                for par in (0, 1):
                    nc.tensor.matmul(
                        uv[64 * par:64 * par + 64, :], W("c0"),
                        a3[:, 512 * par:512 * par + 512],
                        start=False, stop=False, tile_position=(0, 64 * par))
                for par in (0, 1):
                    nc.tensor.matmul(
                        uv[64 * par:64 * par + 64, :], W("c1"),
                        a3[:, 1024 + 512 * par:1024 + 512 * par + 512],
                        start=False, stop=True, tile_position=(0, 64 * par))
                nc.scalar.activation(a5[:, :], uv, AF.Prelu, bias=0.0,
                                     scale=1.0, alpha=SLOPE)

            def emit_t6(st):
                """t6 into a shared [128,1024] psum tile: u6 at [0:256],
                u7 at [256:384], e8/y at [384:512]."""
                a5 = state[st]["a5"]
                u = pu.tile([128, 1024], f32, name="uttt", tag="u")
                state[st]["uttt"] = u
                u6 = u[:, 0:256]
                a6 = ap.tile([128, 256], f16, name="a6", tag="a6")
                t6c = ap.tile([128, 256], f16, name="t6c", tag="t6c")
                s6 = ap.tile([128, 256], f16, name="s6", tag="s6")
                state[st]["a6"], state[st]["s6"] = a6, s6
                for par in (0, 1):
                    nc.tensor.matmul(u6[64 * par:64 * par + 64, :], W("t6"),
                                     a5[:, 256 * par:256 * par + 256],
                                     start=True, stop=True,
                                     tile_position=(0, 64 * par))
                nc.scalar.activation(a6[:, :], u6, AF.Prelu, bias=0.0,
                                     scale=1.0, alpha=SLOPE)
                nc.vector.tensor_scalar(t6c[:, :], u6, sq6_scale, None,
                                        OP.mult)
                nc.gpsimd.tensor_tensor(s6[:, :], t6c[:, :], t6c[:, :],
                                        OP.mult)

            def emit_t7(st):
                u, a6 = state[st]["uttt"], state[st]["a6"]
                u7 = u[:, 256:384]
                a7 = ap.tile([128, 128], f16, name="a7", tag="a7")
                t7c = ap.tile([128, 128], f16, name="t7c", tag="t7c")
                s7 = ap.tile([128, 128], f16, name="s7", tag="s7")
                state[st]["a7"], state[st]["s7"] = a7, s7
                for par in (0, 1):
                    nc.tensor.matmul(u7[64 * par:64 * par + 64, :], W("t7"),
                                     a6[:, 128 * par:128 * par + 128],
                                     start=True, stop=True,
                                     tile_position=(0, 64 * par))
                nc.scalar.activation(a7[:, :], u7, AF.Prelu, bias=0.0,
                                     scale=1.0, alpha=SLOPE)
                nc.vector.tensor_copy(t7c[:, :], u7)
                nc.gpsimd.tensor_tensor(s7[:, :], t7c[:, :], t7c[:, :],
                                        OP.mult)

            def emit_tails(st):
                u = state[st]["uttt"]
                a7, s6, s7 = state[st]["a7"], state[st]["s6"], state[st]["s7"]
                ey = u[:, 384:512]
                # tails: e8 at partitions 0:64, y at 64:128 of ey
                nc.tensor.matmul(ey[0:64, :], W("v7"), s7[:, :],
                                 start=True, stop=False)
                nc.tensor.matmul(ey[0:64, :], W("v6a"), s6[:, 0:128],
                                 start=False, stop=False)
                nc.tensor.matmul(ey[0:64, :], W("v6b"), s6[:, 128:256],
                                 start=False, stop=True)
                nc.tensor.matmul(ey[64:128, :], W("s8"), a7[:, :],
                                 start=True, stop=True, tile_position=(0, 64))
                nc.vector.tensor_copy(
                    outsb[:, st * 128:(st + 1) * 128], u[:, 384:512])
                nc.sync.dma_start(ey_d[:, st * 128:(st + 1) * 128],
                                  outsb[:, st * 128:(st + 1) * 128])
                state[st].clear()

            # Wavefront pipeline across supertiles: stage offsets
            # L1(t), L2(t-1), L3(t-2), t4(t-3), t5(t-4), ttt(t-5).
            # t5 one step after t4 so t5's matmuls never wait on the r4
            # evacuation within the same step (ditto ttt after t5).
            NSTG = 6
            emit_dma(0)
            emit_dma(1)
            for t in range(N_ST + NSTG - 1):
                if t + 2 < N_ST:
                    emit_dma(t + 2)

                def valid(off):
                    return 0 <= t - off < N_ST

                # piece order: dep-chained pieces (t6 -> t7 -> tails feed off
                # same-step ACT/Pool results) are spaced out between ready
                # streams so the PE's 4-deep dependency wait-queue never
                # head-of-line-blocks ready work
                pieces = []
                if valid(1):
                    pieces.append(lambda: emit_l2_chunk(t - 1, 0))
                if valid(0):
                    pieces.append(lambda: emit_l1_chunk(t, 0))
                if valid(3):
                    pieces.append(lambda: emit_t4(t - 3))
                if valid(2):
                    pieces.append(lambda: emit_l3_chunk(t - 2, 0))
                if valid(4):
                    pieces.append(lambda: emit_t5(t - 4))
                if valid(0):
                    pieces.append(lambda: emit_l1_chunk(t, 1))
                if valid(5):
                    pieces.append(lambda: emit_t6(t - 5))
                if valid(1):
                    pieces.append(lambda: emit_l2_chunk(t - 1, 1))
                if valid(5):
                    pieces.append(lambda: emit_t7(t - 5))
                if valid(2):
                    pieces.append(lambda: emit_l3_chunk(t - 2, 1))
                if valid(5):
                    pieces.append(lambda: emit_tails(t - 5))
                for p in pieces:
                    p()
    _split_multi_waits(nc)
    return nc


def kernel(**inputs):
    for l in range(1, 8):
        if np.abs(np.asarray(inputs[f"bt{l}"], np.float32)).max() > 0:
            return _numpy_forward(inputs)
        if np.asarray(inputs[f"g{l}"], np.float32).min() <= 0:
            return _numpy_forward(inputs)

    wpack, offs, wcols, ks = _build_consts(inputs)
    global _WCOLS
    _WCOLS = wcols

    x = np.asarray(inputs["x"], np.float32)
    xT = np.ascontiguousarray(x.T)               # [128, 524288]
    xhi = xT.astype(F16)
    b8 = float(np.asarray(inputs["b8"], np.float32).reshape(-1)[0])
    sq6_scale = float(np.sqrt(EPS) * ks[6])      # (scale*u6)^2 = eps*k7^2*u6^2

    nc = _build_program(offs, wpack.shape[1], sq6_scale)

    in_maps = []
    for c in range(N_CORES):
        s = slice(c * RPC, (c + 1) * RPC)
        in_maps.append({
            "xhi": np.ascontiguousarray(xhi[:, s]),
            "wpack": wpack,
        })

    from concourse.bass_utils import run_bass_kernel_spmd
    res = run_bass_kernel_spmd(nc, in_maps, core_ids=list(range(N_CORES)))

    # device ey layout [128, N_ST*128]: per supertile block [128,128]:
    # partitions 0:64 = E8, 64:128 = y; row-in-core = st*8192 + m*128 + j
    def unpack(arr):
        a = arr.reshape(128, N_ST, 128)
        e8 = a[0:64].transpose(1, 0, 2).reshape(-1)
        y = a[64:128].transpose(1, 0, 2).reshape(-1)
        return e8, y

    out = np.empty((ROWS, 1), np.float32)
    e8r = np.empty(ROWS, np.float64)
    for c in range(N_CORES):
        e8, y = unpack(res.results[c]["ey"].astype(np.float64))
        with np.errstate(invalid="ignore", divide="ignore"):
            out[c * RPC:(c + 1) * RPC, 0] = (y / np.sqrt(e8) + b8)
        e8r[c * RPC:(c + 1) * RPC] = e8

    finite = np.isfinite(e8r) & (e8r > 0)
    med = float(np.median(e8r[finite])) if finite.any() else 0.0
    if not np.isfinite(med) or med <= 0:
        return _numpy_forward(inputs).reshape(ROWS, 1)
    bad = (~finite) | (e8r < FLAG_RATIO * med) | (~np.isfinite(out[:, 0]))
    if bad.mean() > 0.30:
        return _numpy_forward(inputs).reshape(ROWS, 1)
    idx = np.nonzero(bad)[0]
    if idx.size:
        out[idx, 0] = _ref_rows(inputs, idx).astype(np.float32).ravel()
    return out


# revision 38
# speedup vs baseline: 1.0330x; 1.0266x over previous
"""Trainium2 Bass kernel for nn_DiscriminatorModel (8-layer MLP with
LayerNorm+LeakyReLU, 524288x128 input, data-parallel over 8 NeuronCores).

Evacuation-balanced redesign of the previous single-word-fp16 kernel.
Profiling showed the old kernel was bound by PSUM-evacuation work on the
Scalar (ACT) engine (71% busy incl. DMA dispatch) with the PE array only
~45% loaded. Changes:

  - Math identical to before: mean-centering folded into weights host-side,
    per-row rsqrt deferred to the end via E8 = v7' + eps*k7^2*v6', per-layer
    power-of-2 scale calibration, low-E8 rows recomputed on host in float64.
  - L4 now also uses the relu-trick: DVE/ACT evacuate r4 = 0.8*relu(u4) (a
    one-op tensor_scalar / Prelu(alpha=0) split between both engines) and
    the 0.2*u4 linear part rides into L5 via composed stationaries
    C[h] = 0.2 * T4 @ T5[64h:64h+64, :] streaming a3 once more. This takes
    L4's Prelu off the critical ACT queue.
  - t6/t7 fp16 staging on DVE, squares on the (otherwise idle) Pool engine;
    ACT runs a single Prelu table the whole kernel (r4's 0.8*relu is
    Prelu(scale=0.8, alpha=0) - same table).
  - Division by sqrt(E8) moved to the host: the kernel ships raw per-row
    y and E8 ([128,128] psum block per supertile, one DMA each), dropping
    the on-device sqrt/reciprocal/multiply epilogue entirely.
  - All DMA dispatch on the Sync queue (the old kernel burned 13us of ACT
    issuing DMA descriptors).
  - L1 matmuls run on 4 PE column strips (measured 6.8 cols/ns vs 2.2
    serial); evacuation tiles are [128,1024] (ACT 114 Ge/s, DVE 104 Ge/s).

Requires all LayerNorm beta == 0 and gamma > 0 (true for the reference
inputs); otherwise falls back to a float64 numpy forward pass.
"""

import numpy as np

EPS = 1e-5
SLOPE = 0.2
DIMS = [128, 32, 64, 32, 16, 8, 4, 2]
N_CORES = 8
ROWS = 524288
RPC = ROWS // N_CORES        # 65536 rows per core
R_ST = 8192                  # rows per supertile
N_ST = RPC // R_ST           # 8 supertiles per core
F16 = np.float16
SIGMA_T = 16.0               # per-layer target std after scaling
FLAG_RATIO = 6e-2            # host-patch rows with E8 < ratio*median


def _center(W):
    d = W.shape[1]
    return np.asarray(W, np.float64) @ (np.eye(d) - 1.0 / d)


def _blockdiag(W, c):
    din, dout = W.shape
    out = np.zeros((c * din, c * dout), W.dtype)
    for b in range(c):
        out[b * din:(b + 1) * din, b * dout:(b + 1) * dout] = W
    return out


def _pos(c_out):
    """Output-block position map for split transitions: even blocks to the
    low partition half, odd blocks to the high half."""
    return lambda b: (b % 2) * (c_out // 2) + (b // 2)


def _transition_stat_split(W, c_in, pos_in):
    """Stationary for a c_in -> 2*c_in transition in parity-SPLIT layout."""
    din, dout = W.shape
    w = 128 // (2 * c_in)
    assert w == dout
    S = np.zeros((128, 64), np.float64)
    for g in range(c_in):
        rp = pos_in(g) * din
        S[rp:rp + din, g * w:(g + 1) * w] = W
    return S


def _var_stats(g6, g7, pos6, pos7):
    """vpk stationaries for v6 (par a/b) and v7, with 1/(d*g^2) weights."""
    d6, d7 = DIMS[6], DIMS[7]
    w6 = 1.0 / (d6 * np.square(np.asarray(g6, np.float64)))
    w7 = 1.0 / (d7 * np.square(np.asarray(g7, np.float64)))
    V6 = []
    for par in range(2):
        S = np.zeros((128, 64), np.float64)
        for m in range(64):
            if m % 2 != par:
                continue
            g = m // 2
            rp = pos6(g) * d6
            S[rp:rp + d6, m] = w6
        V6.append(S)
    V7 = np.zeros((128, 64), np.float64)
    for m in range(64):
        rp = pos7(m) * d7
        V7[rp:rp + d7, m] = w7
    return V6[0], V6[1], V7


def _ref_rows(inp, idx):
    """float64 reference forward for a subset of rows."""
    h = np.asarray(inp["x"], np.float32)[idx].astype(np.float64)
    for i in range(7):
        W = np.asarray(inp[f"W{i+1}"], np.float32).astype(np.float64)
        gg = np.asarray(inp[f"g{i+1}"], np.float32).astype(np.float64)
        bb = np.asarray(inp[f"bt{i+1}"], np.float32).astype(np.float64)
        h = h @ W
        m = h.mean(-1, keepdims=True)
        v = np.square(h - m).mean(-1, keepdims=True)
        h = (h - m) / np.sqrt(v + EPS) * gg + bb
        h = np.where(h > 0, h, SLOPE * h)
    return (h @ np.asarray(inp["W8"], np.float32).astype(np.float64)
            + np.asarray(inp["b8"], np.float32).astype(np.float64))


def _numpy_forward(inp):
    return _ref_rows(inp, slice(None)).astype(np.float32)


def _calibrate(inp, Sg):
    """Per-layer power-of-2 scales so std(u_l') ~= SIGMA_T."""
    xs = np.asarray(inp["x"], np.float32)[:4096].astype(np.float64)
    ks = []
    h, C = xs, 1.0
    for i in range(7):
        u = h @ Sg[i]
        s = float(u.std())
        if not np.isfinite(s) or s <= 0:
            k = 1.0
        else:
            k = float(2.0 ** np.round(np.log2(SIGMA_T / (C * s))))
        ks.append(k)
        C *= k
        h = np.where(u > 0, u, SLOPE * u)
    return ks


def _build_consts(inp):
    g = [np.asarray(inp[f"g{l}"], np.float32).astype(np.float64)
         for l in range(1, 8)]
    Sg = [_center(np.asarray(inp[f"W{l}"], np.float32)) @ np.diag(g[l - 1])
          for l in range(1, 8)]
    ks = _calibrate(inp, Sg)
    S = [Sg[i] * ks[i] for i in range(7)]
    W8 = np.asarray(inp["W8"], np.float32).astype(np.float64)

    cols = {}
    def add(name, arr):
        cols[name] = arr.astype(F16)

    add("s1", S[0])                                    # [128, 32]
    bd2 = _blockdiag(S[1], 2)                          # [64, 128]
    add("s2", np.vstack([bd2, bd2]))                   # [128, 128]
    s3 = _blockdiag(S[2], 2)                           # [128, 64]
    add("s3", s3)
    sc = 0.2 * (bd2 @ s3)                              # [64, 64]
    add("sc", np.vstack([sc, sc]))                     # [128, 64]
    t4 = _transition_stat_split(S[3], 4, lambda b: b)
    add("t4", t4)
    t5 = _transition_stat_split(S[4], 8, _pos(8))
    add("t5", t5)
    add("t6", _transition_stat_split(S[5], 16, _pos(16)))
    add("t7", _transition_stat_split(S[6], 32, _pos(32)))
    # composed stationaries for the L4 relu-trick (0.2*u4 into L5)
    add("c0", 0.2 * (t4 @ t5[0:64, :]))                # [128, 64]
    add("c1", 0.2 * (t4 @ t5[64:128, :]))              # [128, 64]
    pos6, pos7 = _pos(32), _pos(64)
    s8 = np.zeros((128, 64), np.float64)
    for m in range(64):
        rp = pos7(m) * 2
        s8[rp:rp + 2, m] = W8[:, 0]
    add("s8", s8)                                      # [128, 64]
    V6a, V6b, V7 = _var_stats(g[5], g[6], pos6, pos7)
    add("v6a", V6a)
    add("v6b", V6b)
    add("v7", V7)

    order = sorted(cols.keys())
    offs, total = {}, 0
    wcols = {}
    for k in order:
        offs[k] = total
        wcols[k] = cols[k].shape[1]
        total += cols[k].shape[1]
    wpack = np.zeros((128, total), F16)
    for k in order:
        wpack[:, offs[k]:offs[k] + cols[k].shape[1]] = cols[k]
    return wpack, offs, wcols, ks


def _split_multi_waits(nc):
    """Walrus build limit: <=1 sync wait per instruction. Hoist extras onto
    same-engine NOPs inserted just before the instruction."""
    import concourse.mybir as mybir
    import bass_rust
    cnt = 0
    for f in nc.m.functions:
        for blk in f.blocks:
            newlist = []
            for inst in blk.instructions:
                si = inst.sync_info
                waits = list(si.on_wait) if si is not None and si.on_wait else []
                if len(waits) > 1:
                    for w in waits[:-1]:
                        nop = mybir.InstNoOp(name=f"waitnop_{cnt}", ins=[], outs=[])
                        cnt += 1
                        nop.engine = inst.engine
                        nop.sync_info = bass_rust.SyncInfo(on_wait=[w], on_update=[])
                        newlist.append(nop)
                    inst.sync_info = bass_rust.SyncInfo(
                        on_wait=[waits[-1]], on_update=list(si.on_update))
                newlist.append(inst)
            blk.instructions = newlist
    return cnt


_WCOLS = {}


def _build_program(offs, wpack_cols, sq6_scale):
    import concourse.bass as bass
    import concourse.mybir as mybir
    from concourse.tile import TileContext
    from contextlib import ExitStack

    import bass_rust
    from concourse.tile import TileContext as _TC
    from concourse.vector_clock import ScopedClock

    def _patched_drain(self, tick_clock, wait_clock):
        probe = self.nc.sync.nop()
        wait_clock.add_sem_waits(probe.ins,
                                 ScopedClock({None: tick_clock.global_clock}))
        si = probe.ins.sync_info
        waits = list(si.on_wait) if si is not None else []
        upd = list(si.on_update) if si is not None else []
        probe.ins.sync_info = bass_rust.SyncInfo(on_wait=waits[:1], on_update=upd)
        for w in waits[1:]:
            nop = self.nc.sync.nop()
            nop.ins.sync_info = bass_rust.SyncInfo(on_wait=[w], on_update=[])
        self.nc.sync.drain()
        self.nc.all_engine_barrier()
        assert self.sems is not None
        popped = self.nc._tile_sem_poison_stack.pop()
        assert popped is self._sem_poison
        self.nc.clear_and_free_semaphores(list(self.sems.allocated().values()))
        self.nc.all_engine_barrier()

    _TC._drain_and_barrier = _patched_drain

    f16, f32 = mybir.dt.float16, mybir.dt.float32
    AF = mybir.ActivationFunctionType
    OP = mybir.AluOpType

    nc = bass.Bass(trn_type="TRN2", num_swdge_queues=4)
    xhi_d = nc.dram_tensor("xhi", [128, RPC], f16, kind="ExternalInput")
    wp_d = nc.dram_tensor("wpack", [128, wpack_cols], f16, kind="ExternalInput")
    ey_d = nc.dram_tensor("ey", [128, N_ST * 128], f32, kind="ExternalOutput")

    with TileContext(nc) as tc:
        with ExitStack() as ctx:
            const = ctx.enter_context(tc.tile_pool(name="const", bufs=1))
            # dummy activation with no deps: pulls the ACT table load into
            # the preamble shadow
            warmup = const.tile([128, 1], f32, name="warmup")
            nc.scalar.activation(warmup[:, :], warmup[:, :], AF.Prelu,
                                 bias=0.0, scale=1.0, alpha=SLOPE)
            wp = const.tile([128, wpack_cols], f16)
            nc.sync.dma_start(wp[:, :], wp_d[:, :])

            def W(name):
                return wp[:, offs[name]:offs[name] + _WCOLS[name]]

            xp = ctx.enter_context(tc.tile_pool(name="xp", bufs=12))
            ap = ctx.enter_context(tc.tile_pool(name="ap", bufs=3))
            fin = ctx.enter_context(tc.tile_pool(name="fin", bufs=1))
            pu = ctx.enter_context(tc.tile_pool(name="pu", bufs=2, space="PSUM"))
            pv = ctx.enter_context(tc.tile_pool(name="pv", bufs=2, space="PSUM"))

            outsb = fin.tile([128, N_ST * 128], f32, name="outsb", tag="outsb")

            s2q = [wp[:, offs["s2"]:offs["s2"] + 128][64 * q:64 * (q + 1), :]
                   for q in range(2)]
            scq = [wp[:, offs["sc"]:offs["sc"] + 64][64 * q:64 * (q + 1), :]
                   for q in range(2)]

            state = {}

            def emit_dma(st):
                x0 = st * R_ST
                xch = []
                for kk in range(4):
                    t = xp.tile([128, 2048], f16, name=f"xc{kk}", tag="xc")
                    nc.sync.dma_start(
                        t[:, :], xhi_d[:, x0 + 2048 * kk:x0 + 2048 * (kk + 1)])
                    xch.append(t)
                state[st] = {"xch": xch}

            def emit_l1_chunk(st, c):
                xch = state[st]["xch"]
                if c == 0:
                    state[st]["a1"] = ap.tile([128, 2048], f16, name="a1",
                                              tag="a1")
                a1 = state[st]["a1"]
                u = pu.tile([128, 1024], f32, name="u", tag="u")
                if st == 0:
                    # ramp: consume DMA chunks in arrival order (b-outer)
                    for b in range(4):
                        for o in (0, 512):
                            rhs = xch[b][:, 1024 * c + o:1024 * c + o + 512]
                            nc.tensor.matmul(
                                u[32 * b:32 * (b + 1), o:o + 512], W("s1"), rhs,
                                start=True, stop=True, tile_position=(0, 32 * b))
                else:
                    for o in (0, 512):
                        for b in range(4):
                            rhs = xch[b][:, 1024 * c + o:1024 * c + o + 512]
                            nc.tensor.matmul(
                                u[32 * b:32 * (b + 1), o:o + 512], W("s1"), rhs,
                                start=True, stop=True, tile_position=(0, 32 * b))
                nc.scalar.activation(a1[:, 1024 * c:1024 * (c + 1)], u[:, :],
                                     AF.Prelu, bias=0.0, scale=1.0,
                                     alpha=SLOPE)

            def emit_l2_chunk(st, c):
                a1 = state[st]["a1"]
                if c == 0:
                    state[st]["r2"] = [
                        ap.tile([128, 2048], f16, name=f"r2{q}", tag=f"r2{q}")
                        for q in range(2)]
                r2 = state[st]["r2"]
                for q in range(2):
                    us = pv.tile([128, 1024], f32, name="uv2", tag="v")
                    for o in (0, 512):
                        rhs = a1[64 * q:64 * (q + 1),
                                 1024 * c + o:1024 * c + o + 512]
                        nc.tensor.matmul(
                            us[:, o:o + 512], s2q[q], rhs,
                            start=True, stop=True, tile_position=(64 * q, 0))
                    nc.vector.tensor_scalar(
                        r2[q][:, 1024 * c:1024 * (c + 1)], us[:, :],
                        0.8, 0.0, OP.mult, OP.max)

            def emit_l3_chunk(st, c):
                a1, r2 = state[st]["a1"], state[st]["r2"]
                if c == 0:
                    state[st]["a3"] = ap.tile([128, 2048], f16, name="a3",
                                              tag="a3")
                a3 = state[st]["a3"]
                u = pv.tile([128, 1024], f32, name="u3", tag="v")
                # all s3 matmuls first, then all composed ones: adjacent
                # queue entries target independent regions so the PE can
                # overlap them (s3->sc accumulation pairs would serialize)
                for o in (0, 512):
                    for q in range(2):
                        nc.tensor.matmul(
                            u[64 * q:64 * (q + 1), o:o + 512], W("s3"),
                            r2[q][:, 1024 * c + o:1024 * c + o + 512],
                            start=True, stop=False, tile_position=(0, 64 * q))
                for o in (0, 512):
                    for q in range(2):
                        nc.tensor.matmul(
                            u[64 * q:64 * (q + 1), o:o + 512], scq[q],
                            a1[64 * q:64 * (q + 1),
                               1024 * c + o:1024 * c + o + 512],
                            start=False, stop=True,
                            tile_position=(64 * q, 64 * q))
                nc.scalar.activation(a3[:, 1024 * c:1024 * (c + 1)], u[:, :],
                                     AF.Prelu, bias=0.0, scale=1.0,
                                     alpha=SLOPE)

            def emit_t4(st):
                a3 = state[st]["a3"]
                r4 = ap.tile([128, 1024], f16, name="r4", tag="r4")
                state[st]["r4"] = r4
                u = pu.tile([128, 1024], f32, name="u4", tag="u")
                for par in (0, 1):
                    for o in (0, 512):
                        nc.tensor.matmul(
                            u[64 * par:64 * par + 64, o:o + 512], W("t4"),
                            a3[:, 1024 * par + o:1024 * par + o + 512],
                            start=True, stop=True, tile_position=(0, 64 * par))
                # r4 = 0.8*relu(u4), all on DVE: ACT is the step clock
                nc.vector.tensor_scalar(r4[:, :], u[:, :],
                                        0.8, 0.0, OP.mult, OP.max)

            def emit_t5(st):
                a3, r4 = state[st]["a3"], state[st]["r4"]
                a5 = ap.tile([128, 512], f16, name="a5", tag="a5")
                state[st]["a5"] = a5
                u = pu.tile([128, 1024], f32, name="u5", tag="u")
                uv = u[:, 0:512]
                # group by stationary (not by parity): parity pairs are on
                # independent strips and can overlap in the array
                for par in (0, 1):
                    nc.tensor.matmul(uv[64 * par:64 * par + 64, :], W("t5"),
                                     r4[:, 512 * par:512 * par + 512],
                                     start=True, stop=False,
                                     tile_position=(0, 64 * par))
                for par in (0, 1):
                    nc.tensor.matmul(
                        uv[64 * par:64 * par + 64, :], W("c0"),
                        a3[:, 512 * par:512 * par + 512],
                        start=False, stop=False, tile_position=(0, 64 * par))
                for par in (0, 1):
                    nc.tensor.matmul(
                        uv[64 * par:64 * par + 64, :], W("c1"),
                        a3[:, 1024 + 512 * par:1024 + 512 * par + 512],
                        start=False, stop=True, tile_position=(0, 64 * par))
                nc.scalar.activation(a5[:, :], uv, AF.Prelu, bias=0.0,
                                     scale=1.0, alpha=SLOPE)

            def emit_t6(st):
                """t6 into a shared [128,1024] psum tile: u6 at [0:256],
                u7 at [256:384], e8/y at [384:512]."""
                a5 = state[st]["a5"]
                u = pu.tile([128, 1024], f32, name="uttt", tag="u")
                state[st]["uttt"] = u
                u6 = u[:, 0:256]
                a6 = ap.tile([128, 256], f16, name="a6", tag="a6")
                t6c = ap.tile([128, 256], f16, name="t6c", tag="t6c")
                s6 = ap.tile([128, 256], f16, name="s6", tag="s6")
                state[st]["a6"], state[st]["s6"] = a6, s6
                for par in (0, 1):
                    nc.tensor.matmul(u6[64 * par:64 * par + 64, :], W("t6"),
                                     a5[:, 256 * par:256 * par + 256],
                                     start=True, stop=True,
                                     tile_position=(0, 64 * par))
                nc.scalar.activation(a6[:, :], u6, AF.Prelu, bias=0.0,
                                     scale=1.0, alpha=SLOPE)
                nc.vector.tensor_scalar(t6c[:, :], u6, sq6_scale, None,
                                        OP.mult)
                nc.gpsimd.tensor_tensor(s6[:, :], t6c[:, :], t6c[:, :],
                                        OP.mult)

            def emit_t7(st):
                u, a6 = state[st]["uttt"], state[st]["a6"]
                u7 = u[:, 256:384]
                a7 = ap.tile([128, 128], f16, name="a7", tag="a7")
                t7c = ap.tile([128, 128], f16, name="t7c", tag="t7c")
                s7 = ap.tile([128, 128], f16, name="s7", tag="s7")
                state[st]["a7"], state[st]["s7"] = a7, s7
                for par in (0, 1):
                    nc.tensor.matmul(u7[64 * par:64 * par + 64, :], W("t7"),
                                     a6[:, 128 * par:128 * par + 128],
                                     start=True, stop=True,
                                     tile_position=(0, 64 * par))
                nc.scalar.activation(a7[:, :], u7, AF.Prelu, bias=0.0,
                                     scale=1.0, alpha=SLOPE)
                nc.vector.tensor_copy(t7c[:, :], u7)
                nc.gpsimd.tensor_tensor(s7[:, :], t7c[:, :], t7c[:, :],
                                        OP.mult)

            def emit_tails(st):
                u = state[st]["uttt"]
                a7, s6, s7 = state[st]["a7"], state[st]["s6"], state[st]["s7"]
                ey = u[:, 384:512]
                # tails: e8 at partitions 0:64, y at 64:128 of ey
                nc.tensor.matmul(ey[0:64, :], W("v7"), s7[:, :],
                                 start=True, stop=False)
                nc.tensor.matmul(ey[0:64, :], W("v6a"), s6[:, 0:128],
                                 start=False, stop=False)
                nc.tensor.matmul(ey[0:64, :], W("v6b"), s6[:, 128:256],
                                 start=False, stop=True)
                nc.tensor.matmul(ey[64:128, :], W("s8"), a7[:, :],
                                 start=True, stop=True, tile_position=(0, 64))
                nc.vector.tensor_copy(
                    outsb[:, st * 128:(st + 1) * 128], u[:, 384:512])
                nc.sync.dma_start(ey_d[:, st * 128:(st + 1) * 128],
                                  outsb[:, st * 128:(st + 1) * 128])
                state[st].clear()

            # Wavefront pipeline across supertiles: stage offsets
            # L1(t), L2(t-1), L3(t-2), t4(t-3), t5(t-4), ttt(t-5).
            # t5 one step after t4 so t5's matmuls never wait on the r4
            # evacuation within the same step (ditto ttt after t5).
            NSTG = 6
            emit_dma(0)
            emit_dma(1)
            for t in range(N_ST + NSTG - 1):
                if t + 2 < N_ST:
                    emit_dma(t + 2)

                LAST = N_ST - 1

                def valid(off):
                    s = t - off
                    # the last supertile's deep stages run compressed (below)
                    return 0 <= s < N_ST and not (s == LAST and off >= 2)

                # piece order: dep-chained pieces (t6 -> t7 -> tails feed off
                # same-step ACT/Pool results) are spaced out between ready
                # streams so the PE's 4-deep dependency wait-queue never
                # head-of-line-blocks ready work
                pieces = []
                if valid(1):
                    pieces.append(lambda: emit_l2_chunk(t - 1, 0))
                if valid(0):
                    pieces.append(lambda: emit_l1_chunk(t, 0))
                if valid(3):
                    pieces.append(lambda: emit_t4(t - 3))
                if valid(2):
                    pieces.append(lambda: emit_l3_chunk(t - 2, 0))
                if valid(4):
                    pieces.append(lambda: emit_t5(t - 4))
                if valid(0):
                    pieces.append(lambda: emit_l1_chunk(t, 1))
                if valid(5):
                    pieces.append(lambda: emit_t6(t - 5))
                if valid(1):
                    pieces.append(lambda: emit_l2_chunk(t - 1, 1))
                if valid(5):
                    pieces.append(lambda: emit_t7(t - 5))
                if valid(2):
                    pieces.append(lambda: emit_l3_chunk(t - 2, 1))
                if valid(5):
                    pieces.append(lambda: emit_tails(t - 5))
                # drain compression: the last supertile's stages chain with
                # tighter offsets (drain steps have idle queues, so the
                # same-step dependencies cost latency, not throughput)
                if t == LAST + 1:
                    pieces.append(lambda: emit_l3_chunk(LAST, 0))
                    pieces.append(lambda: emit_l3_chunk(LAST, 1))
                if t == LAST + 2:
                    pieces.append(lambda: emit_t4(LAST))
                    pieces.append(lambda: emit_t5(LAST))
                if t == LAST + 3:
                    pieces.append(lambda: emit_t6(LAST))
                    pieces.append(lambda: emit_t7(LAST))
                    pieces.append(lambda: emit_tails(LAST))
                for p in pieces:
                    p()
    _split_multi_waits(nc)
    return nc


def kernel(**inputs):
    for l in range(1, 8):
        if np.abs(np.asarray(inputs[f"bt{l}"], np.float32)).max() > 0:
            return _numpy_forward(inputs)
        if np.asarray(inputs[f"g{l}"], np.float32).min() <= 0:
            return _numpy_forward(inputs)

    wpack, offs, wcols, ks = _build_consts(inputs)
    global _WCOLS
    _WCOLS = wcols

    x = np.asarray(inputs["x"], np.float32)
    xT = np.ascontiguousarray(x.T)               # [128, 524288]
    xhi = xT.astype(F16)
    b8 = float(np.asarray(inputs["b8"], np.float32).reshape(-1)[0])
    sq6_scale = float(np.sqrt(EPS) * ks[6])      # (scale*u6)^2 = eps*k7^2*u6^2

    nc = _build_program(offs, wpack.shape[1], sq6_scale)

    in_maps = []
    for c in range(N_CORES):
        s = slice(c * RPC, (c + 1) * RPC)
        in_maps.append({
            "xhi": np.ascontiguousarray(xhi[:, s]),
            "wpack": wpack,
        })

    from concourse.bass_utils import run_bass_kernel_spmd
    res = run_bass_kernel_spmd(nc, in_maps, core_ids=list(range(N_CORES)))

    # device ey layout [128, N_ST*128]: per supertile block [128,128]:
    # partitions 0:64 = E8, 64:128 = y; row-in-core = st*8192 + m*128 + j
    def unpack(arr):
        a = arr.reshape(128, N_ST, 128)
        e8 = a[0:64].transpose(1, 0, 2).reshape(-1)
        y = a[64:128].transpose(1, 0, 2).reshape(-1)
        return e8, y

    out = np.empty((ROWS, 1), np.float32)
    e8r = np.empty(ROWS, np.float64)
    for c in range(N_CORES):
        e8, y = unpack(res.results[c]["ey"].astype(np.float64))
        with np.errstate(invalid="ignore", divide="ignore"):
            out[c * RPC:(c + 1) * RPC, 0] = (y / np.sqrt(e8) + b8)
        e8r[c * RPC:(c + 1) * RPC] = e8

    finite = np.isfinite(e8r) & (e8r > 0)
    med = float(np.median(e8r[finite])) if finite.any() else 0.0
    if not np.isfinite(med) or med <= 0:
        return _numpy_forward(inputs).reshape(ROWS, 1)
    bad = (~finite) | (e8r < FLAG_RATIO * med) | (~np.isfinite(out[:, 0]))
    if bad.mean() > 0.30:
        return _numpy_forward(inputs).reshape(ROWS, 1)
    idx = np.nonzero(bad)[0]
    if idx.size:
        out[idx, 0] = _ref_rows(inputs, idx).astype(np.float32).ravel()
    return out


# revision 39
# speedup vs baseline: 1.0563x; 1.0226x over previous
"""Trainium2 Bass kernel for nn_DiscriminatorModel (8-layer MLP with
LayerNorm+LeakyReLU, 524288x128 input, data-parallel over 8 NeuronCores).

Evacuation-balanced redesign of the previous single-word-fp16 kernel.
Profiling showed the old kernel was bound by PSUM-evacuation work on the
Scalar (ACT) engine (71% busy incl. DMA dispatch) with the PE array only
~45% loaded. Changes:

  - Math identical to before: mean-centering folded into weights host-side,
    per-row rsqrt deferred to the end via E8 = v7' + eps*k7^2*v6', per-layer
    power-of-2 scale calibration, low-E8 rows recomputed on host in float64.
  - L4 now also uses the relu-trick: DVE/ACT evacuate r4 = 0.8*relu(u4) (a
    one-op tensor_scalar / Prelu(alpha=0) split between both engines) and
    the 0.2*u4 linear part rides into L5 via composed stationaries
    C[h] = 0.2 * T4 @ T5[64h:64h+64, :] streaming a3 once more. This takes
    L4's Prelu off the critical ACT queue.
  - t6/t7 fp16 staging on DVE, squares on the (otherwise idle) Pool engine;
    ACT runs a single Prelu table the whole kernel (r4's 0.8*relu is
    Prelu(scale=0.8, alpha=0) - same table).
  - Division by sqrt(E8) moved to the host: the kernel ships raw per-row
    y and E8 ([128,128] psum block per supertile, one DMA each), dropping
    the on-device sqrt/reciprocal/multiply epilogue entirely.
  - All DMA dispatch on the Sync queue (the old kernel burned 13us of ACT
    issuing DMA descriptors).
  - L1 matmuls run on 4 PE column strips (measured 6.8 cols/ns vs 2.2
    serial); evacuation tiles are [128,1024] (ACT 114 Ge/s, DVE 104 Ge/s).

Requires all LayerNorm beta == 0 and gamma > 0 (true for the reference
inputs); otherwise falls back to a float64 numpy forward pass.
"""

import numpy as np

EPS = 1e-5
SLOPE = 0.2
DIMS = [128, 32, 64, 32, 16, 8, 4, 2]
N_CORES = 8
ROWS = 524288
RPC = ROWS // N_CORES        # 65536 rows per core
R_ST = 8192                  # rows per supertile
N_ST = RPC // R_ST           # 8 supertiles per core
F16 = np.float16
SIGMA_T = 16.0               # per-layer target std after scaling
FLAG_RATIO = 6e-2            # host-patch rows with E8 < ratio*median


def _center(W):
    d = W.shape[1]
    return np.asarray(W, np.float64) @ (np.eye(d) - 1.0 / d)


def _blockdiag(W, c):
    din, dout = W.shape
    out = np.zeros((c * din, c * dout), W.dtype)
    for b in range(c):
        out[b * din:(b + 1) * din, b * dout:(b + 1) * dout] = W
    return out


def _pos(c_out):
    """Output-block position map for split transitions: even blocks to the
    low partition half, odd blocks to the high half."""
    return lambda b: (b % 2) * (c_out // 2) + (b // 2)


def _transition_stat_split(W, c_in, pos_in):
    """Stationary for a c_in -> 2*c_in transition in parity-SPLIT layout."""
    din, dout = W.shape
    w = 128 // (2 * c_in)
    assert w == dout
    S = np.zeros((128, 64), np.float64)
    for g in range(c_in):
        rp = pos_in(g) * din
        S[rp:rp + din, g * w:(g + 1) * w] = W
    return S


def _var_stats(g6, g7, pos6, pos7):
    """vpk stationaries for v6 (par a/b) and v7, with 1/(d*g^2) weights."""
    d6, d7 = DIMS[6], DIMS[7]
    w6 = 1.0 / (d6 * np.square(np.asarray(g6, np.float64)))
    w7 = 1.0 / (d7 * np.square(np.asarray(g7, np.float64)))
    V6 = []
    for par in range(2):
        S = np.zeros((128, 64), np.float64)
        for m in range(64):
            if m % 2 != par:
                continue
            g = m // 2
            rp = pos6(g) * d6
            S[rp:rp + d6, m] = w6
        V6.append(S)
    V7 = np.zeros((128, 64), np.float64)
    for m in range(64):
        rp = pos7(m) * d7
        V7[rp:rp + d7, m] = w7
    return V6[0], V6[1], V7


def _ref_rows(inp, idx):
    """float64 reference forward for a subset of rows."""
    h = np.asarray(inp["x"], np.float32)[idx].astype(np.float64)
    for i in range(7):
        W = np.asarray(inp[f"W{i+1}"], np.float32).astype(np.float64)
        gg = np.asarray(inp[f"g{i+1}"], np.float32).astype(np.float64)
        bb = np.asarray(inp[f"bt{i+1}"], np.float32).astype(np.float64)
        h = h @ W
        m = h.mean(-1, keepdims=True)
        v = np.square(h - m).mean(-1, keepdims=True)
        h = (h - m) / np.sqrt(v + EPS) * gg + bb
        h = np.where(h > 0, h, SLOPE * h)
    return (h @ np.asarray(inp["W8"], np.float32).astype(np.float64)
            + np.asarray(inp["b8"], np.float32).astype(np.float64))


def _numpy_forward(inp):
    return _ref_rows(inp, slice(None)).astype(np.float32)


def _calibrate(inp, Sg):
    """Per-layer power-of-2 scales so std(u_l') ~= SIGMA_T."""
    xs = np.asarray(inp["x"], np.float32)[:4096].astype(np.float64)
    ks = []
    h, C = xs, 1.0
    for i in range(7):
        u = h @ Sg[i]
        s = float(u.std())
        if not np.isfinite(s) or s <= 0:
            k = 1.0
        else:
            k = float(2.0 ** np.round(np.log2(SIGMA_T / (C * s))))
        ks.append(k)
        C *= k
        h = np.where(u > 0, u, SLOPE * u)
    return ks


def _build_consts(inp):
    g = [np.asarray(inp[f"g{l}"], np.float32).astype(np.float64)
         for l in range(1, 8)]
    Sg = [_center(np.asarray(inp[f"W{l}"], np.float32)) @ np.diag(g[l - 1])
          for l in range(1, 8)]
    ks = _calibrate(inp, Sg)
    S = [Sg[i] * ks[i] for i in range(7)]
    W8 = np.asarray(inp["W8"], np.float32).astype(np.float64)

    cols = {}
    def add(name, arr):
        cols[name] = arr.astype(F16)

    add("s1", S[0])                                    # [128, 32]
    bd2 = _blockdiag(S[1], 2)                          # [64, 128]
    add("s2", np.vstack([bd2, bd2]))                   # [128, 128]
    s3 = _blockdiag(S[2], 2)                           # [128, 64]
    add("s3", s3)
    sc = 0.2 * (bd2 @ s3)                              # [64, 64]
    add("sc", np.vstack([sc, sc]))                     # [128, 64]
    t4 = _transition_stat_split(S[3], 4, lambda b: b)
    add("t4", t4)
    t5 = _transition_stat_split(S[4], 8, _pos(8))
    add("t5", t5)
    add("t6", _transition_stat_split(S[5], 16, _pos(16)))
    add("t7", _transition_stat_split(S[6], 32, _pos(32)))
    # composed stationaries for the L4 relu-trick (0.2*u4 into L5)
    add("c0", 0.2 * (t4 @ t5[0:64, :]))                # [128, 64]
    add("c1", 0.2 * (t4 @ t5[64:128, :]))              # [128, 64]
    pos6, pos7 = _pos(32), _pos(64)
    s8 = np.zeros((128, 64), np.float64)
    for m in range(64):
        rp = pos7(m) * 2
        s8[rp:rp + 2, m] = W8[:, 0]
    add("s8", s8)                                      # [128, 64]
    V6a, V6b, V7 = _var_stats(g[5], g[6], pos6, pos7)
    add("v6a", V6a)
    add("v6b", V6b)
    add("v7", V7)

    order = sorted(cols.keys())
    offs, total = {}, 0
    wcols = {}
    for k in order:
        offs[k] = total
        wcols[k] = cols[k].shape[1]
        total += cols[k].shape[1]
    wpack = np.zeros((128, total), F16)
    for k in order:
        wpack[:, offs[k]:offs[k] + cols[k].shape[1]] = cols[k]
    return wpack, offs, wcols, ks


def _split_multi_waits(nc):
    """Walrus build limit: <=1 sync wait per instruction. Hoist extras onto
    same-engine NOPs inserted just before the instruction."""
    import concourse.mybir as mybir
    import bass_rust
    cnt = 0
    for f in nc.m.functions:
        for blk in f.blocks:
            newlist = []
            for inst in blk.instructions:
                si = inst.sync_info
                waits = list(si.on_wait) if si is not None and si.on_wait else []
                if len(waits) > 1:
                    for w in waits[:-1]:
                        nop = mybir.InstNoOp(name=f"waitnop_{cnt}", ins=[], outs=[])
                        cnt += 1
                        nop.engine = inst.engine
                        nop.sync_info = bass_rust.SyncInfo(on_wait=[w], on_update=[])
                        newlist.append(nop)
                    inst.sync_info = bass_rust.SyncInfo(
                        on_wait=[waits[-1]], on_update=list(si.on_update))
                newlist.append(inst)
            blk.instructions = newlist
    return cnt


_WCOLS = {}


def _build_program(offs, wpack_cols, sq6_scale):
    import concourse.bass as bass
    import concourse.mybir as mybir
    from concourse.tile import TileContext
    from contextlib import ExitStack

    import bass_rust
    from concourse.tile import TileContext as _TC
    from concourse.vector_clock import ScopedClock

    def _patched_drain(self, tick_clock, wait_clock):
        probe = self.nc.sync.nop()
        wait_clock.add_sem_waits(probe.ins,
                                 ScopedClock({None: tick_clock.global_clock}))
        si = probe.ins.sync_info
        waits = list(si.on_wait) if si is not None else []
        upd = list(si.on_update) if si is not None else []
        probe.ins.sync_info = bass_rust.SyncInfo(on_wait=waits[:1], on_update=upd)
        for w in waits[1:]:
            nop = self.nc.sync.nop()
            nop.ins.sync_info = bass_rust.SyncInfo(on_wait=[w], on_update=[])
        self.nc.sync.drain()
        self.nc.all_engine_barrier()
        assert self.sems is not None
        popped = self.nc._tile_sem_poison_stack.pop()
        assert popped is self._sem_poison
        self.nc.clear_and_free_semaphores(list(self.sems.allocated().values()))
        self.nc.all_engine_barrier()

    _TC._drain_and_barrier = _patched_drain

    f16, f32 = mybir.dt.float16, mybir.dt.float32
    AF = mybir.ActivationFunctionType
    OP = mybir.AluOpType

    nc = bass.Bass(trn_type="TRN2", num_swdge_queues=4)
    xhi_d = nc.dram_tensor("xhi", [128, RPC], f16, kind="ExternalInput")
    wp_d = nc.dram_tensor("wpack", [128, wpack_cols], f16, kind="ExternalInput")
    ey_d = nc.dram_tensor("ey", [128, N_ST * 128], f32, kind="ExternalOutput")

    with TileContext(nc) as tc:
        with ExitStack() as ctx:
            const = ctx.enter_context(tc.tile_pool(name="const", bufs=1))
            # dummy activation with no deps: pulls the ACT table load into
            # the preamble shadow
            warmup = const.tile([128, 1], f32, name="warmup")
            nc.scalar.activation(warmup[:, :], warmup[:, :], AF.Prelu,
                                 bias=0.0, scale=1.0, alpha=SLOPE)
            wp = const.tile([128, wpack_cols], f16)
            nc.sync.dma_start(wp[:, :], wp_d[:, :])

            def W(name):
                return wp[:, offs[name]:offs[name] + _WCOLS[name]]

            xp = ctx.enter_context(tc.tile_pool(name="xp", bufs=12))
            ap = ctx.enter_context(tc.tile_pool(name="ap", bufs=3))
            fin = ctx.enter_context(tc.tile_pool(name="fin", bufs=1))
            pu = ctx.enter_context(tc.tile_pool(name="pu", bufs=2, space="PSUM"))
            pv = ctx.enter_context(tc.tile_pool(name="pv", bufs=2, space="PSUM"))

            outsb = fin.tile([128, N_ST * 128], f32, name="outsb", tag="outsb")

            s2q = [wp[:, offs["s2"]:offs["s2"] + 128][64 * q:64 * (q + 1), :]
                   for q in range(2)]
            scq = [wp[:, offs["sc"]:offs["sc"] + 64][64 * q:64 * (q + 1), :]
                   for q in range(2)]

            state = {}

            def emit_dma(st):
                x0 = st * R_ST
                xch = []
                for kk in range(4):
                    t = xp.tile([128, 2048], f16, name=f"xc{kk}", tag="xc")
                    nc.sync.dma_start(
                        t[:, :], xhi_d[:, x0 + 2048 * kk:x0 + 2048 * (kk + 1)])
                    xch.append(t)
                state[st] = {"xch": xch}

            def emit_l1_chunk(st, c):
                xch = state[st]["xch"]
                if c == 0:
                    state[st]["a1"] = ap.tile([128, 2048], f16, name="a1",
                                              tag="a1")
                a1 = state[st]["a1"]
                u = pu.tile([128, 1024], f32, name="u", tag="u")
                if st == 0:
                    # ramp: consume DMA chunks in arrival order (b-outer)
                    for b in range(4):
                        for o in (0, 512):
                            rhs = xch[b][:, 1024 * c + o:1024 * c + o + 512]
                            nc.tensor.matmul(
                                u[32 * b:32 * (b + 1), o:o + 512], W("s1"), rhs,
                                start=True, stop=True, tile_position=(0, 32 * b))
                else:
                    for o in (0, 512):
                        for b in range(4):
                            rhs = xch[b][:, 1024 * c + o:1024 * c + o + 512]
                            nc.tensor.matmul(
                                u[32 * b:32 * (b + 1), o:o + 512], W("s1"), rhs,
                                start=True, stop=True, tile_position=(0, 32 * b))
                nc.scalar.activation(a1[:, 1024 * c:1024 * (c + 1)], u[:, :],
                                     AF.Prelu, bias=0.0, scale=1.0,
                                     alpha=SLOPE)

            def emit_l2_chunk(st, c):
                a1 = state[st]["a1"]
                if c == 0:
                    state[st]["r2"] = [
                        ap.tile([128, 2048], f16, name=f"r2{q}", tag=f"r2{q}")
                        for q in range(2)]
                r2 = state[st]["r2"]
                for q in range(2):
                    us = pv.tile([128, 1024], f32, name="uv2", tag="v")
                    for o in (0, 512):
                        rhs = a1[64 * q:64 * (q + 1),
                                 1024 * c + o:1024 * c + o + 512]
                        nc.tensor.matmul(
                            us[:, o:o + 512], s2q[q], rhs,
                            start=True, stop=True, tile_position=(64 * q, 0))
                    nc.vector.tensor_scalar(
                        r2[q][:, 1024 * c:1024 * (c + 1)], us[:, :],
                        0.8, 0.0, OP.mult, OP.max)

            def emit_l3_chunk(st, c):
                a1, r2 = state[st]["a1"], state[st]["r2"]
                if c == 0:
                    state[st]["a3"] = ap.tile([128, 2048], f16, name="a3",
                                              tag="a3")
                a3 = state[st]["a3"]
                u = pv.tile([128, 1024], f32, name="u3", tag="v")
                # all s3 matmuls first, then all composed ones: adjacent
                # queue entries target independent regions so the PE can
                # overlap them (s3->sc accumulation pairs would serialize)
                for o in (0, 512):
                    for q in range(2):
                        nc.tensor.matmul(
                            u[64 * q:64 * (q + 1), o:o + 512], W("s3"),
                            r2[q][:, 1024 * c + o:1024 * c + o + 512],
                            start=True, stop=False, tile_position=(0, 64 * q))
                for o in (0, 512):
                    for q in range(2):
                        nc.tensor.matmul(
                            u[64 * q:64 * (q + 1), o:o + 512], scq[q],
                            a1[64 * q:64 * (q + 1),
                               1024 * c + o:1024 * c + o + 512],
                            start=False, stop=True,
                            tile_position=(64 * q, 64 * q))
                nc.scalar.activation(a3[:, 1024 * c:1024 * (c + 1)], u[:, :],
                                     AF.Prelu, bias=0.0, scale=1.0,
                                     alpha=SLOPE)

            def emit_t4(st):
                a3 = state[st]["a3"]
                r4 = ap.tile([128, 1024], f16, name="r4", tag="r4")
                state[st]["r4"] = r4
                u = pu.tile([128, 1024], f32, name="u4", tag="u")
                for par in (0, 1):
                    for o in (0, 512):
                        nc.tensor.matmul(
                            u[64 * par:64 * par + 64, o:o + 512], W("t4"),
                            a3[:, 1024 * par + o:1024 * par + o + 512],
                            start=True, stop=True, tile_position=(0, 64 * par))
                # r4 = 0.8*relu(u4), all on DVE: ACT is the step clock
                nc.vector.tensor_scalar(r4[:, :], u[:, :],
                                        0.8, 0.0, OP.mult, OP.max)

            def emit_t5(st):
                a3, r4 = state[st]["a3"], state[st]["r4"]
                a5 = ap.tile([128, 512], f16, name="a5", tag="a5")
                state[st]["a5"] = a5
                u = pu.tile([128, 1024], f32, name="u5", tag="u")
                uv = u[:, 0:512]
                # group by stationary (not by parity): parity pairs are on
                # independent strips and can overlap in the array
                for par in (0, 1):
                    nc.tensor.matmul(uv[64 * par:64 * par + 64, :], W("t5"),
                                     r4[:, 512 * par:512 * par + 512],
                                     start=True, stop=False,
                                     tile_position=(0, 64 * par))
                for par in (0, 1):
                    nc.tensor.matmul(
                        uv[64 * par:64 * par + 64, :], W("c0"),
                        a3[:, 512 * par:512 * par + 512],
                        start=False, stop=False, tile_position=(0, 64 * par))
                for par in (0, 1):
                    nc.tensor.matmul(
                        uv[64 * par:64 * par + 64, :], W("c1"),
                        a3[:, 1024 + 512 * par:1024 + 512 * par + 512],
                        start=False, stop=True, tile_position=(0, 64 * par))
                nc.scalar.activation(a5[:, :], uv, AF.Prelu, bias=0.0,
                                     scale=1.0, alpha=SLOPE)

            def emit_t6(st):
                """t6 into a shared [128,1024] psum tile: u6 at [0:256],
                u7 at [256:384], e8/y at [384:512]."""
                a5 = state[st]["a5"]
                u = pu.tile([128, 1024], f32, name="uttt", tag="u")
                state[st]["uttt"] = u
                u6 = u[:, 0:256]
                a6 = ap.tile([128, 256], f16, name="a6", tag="a6")
                t6c = ap.tile([128, 256], f16, name="t6c", tag="t6c")
                s6 = ap.tile([128, 256], f16, name="s6", tag="s6")
                state[st]["a6"], state[st]["s6"] = a6, s6
                for par in (0, 1):
                    nc.tensor.matmul(u6[64 * par:64 * par + 64, :], W("t6"),
                                     a5[:, 256 * par:256 * par + 256],
                                     start=True, stop=True,
                                     tile_position=(0, 64 * par))
                nc.scalar.activation(a6[:, :], u6, AF.Prelu, bias=0.0,
                                     scale=1.0, alpha=SLOPE)
                nc.vector.tensor_scalar(t6c[:, :], u6, sq6_scale, None,
                                        OP.mult)
                nc.gpsimd.tensor_tensor(s6[:, :], t6c[:, :], t6c[:, :],
                                        OP.mult)

            def emit_t7(st):
                u, a6 = state[st]["uttt"], state[st]["a6"]
                u7 = u[:, 256:384]
                a7 = ap.tile([128, 128], f16, name="a7", tag="a7")
                t7c = ap.tile([128, 128], f16, name="t7c", tag="t7c")
                s7 = ap.tile([128, 128], f16, name="s7", tag="s7")
                state[st]["a7"], state[st]["s7"] = a7, s7
                for par in (0, 1):
                    nc.tensor.matmul(u7[64 * par:64 * par + 64, :], W("t7"),
                                     a6[:, 128 * par:128 * par + 128],
                                     start=True, stop=True,
                                     tile_position=(0, 64 * par))
                nc.scalar.activation(a7[:, :], u7, AF.Prelu, bias=0.0,
                                     scale=1.0, alpha=SLOPE)
                nc.vector.tensor_copy(t7c[:, :], u7)
                nc.gpsimd.tensor_tensor(s7[:, :], t7c[:, :], t7c[:, :],
                                        OP.mult)

            def emit_tails(st):
                u = state[st]["uttt"]
                a7, s6, s7 = state[st]["a7"], state[st]["s6"], state[st]["s7"]
                ey = u[:, 384:512]
                # tails: e8 at partitions 0:64, y at 64:128 of ey
                nc.tensor.matmul(ey[0:64, :], W("v7"), s7[:, :],
                                 start=True, stop=False)
                nc.tensor.matmul(ey[0:64, :], W("v6a"), s6[:, 0:128],
                                 start=False, stop=False)
                nc.tensor.matmul(ey[0:64, :], W("v6b"), s6[:, 128:256],
                                 start=False, stop=True)
                nc.tensor.matmul(ey[64:128, :], W("s8"), a7[:, :],
                                 start=True, stop=True, tile_position=(0, 64))
                nc.vector.tensor_copy(
                    outsb[:, st * 128:(st + 1) * 128], u[:, 384:512])
                nc.sync.dma_start(ey_d[:, st * 128:(st + 1) * 128],
                                  outsb[:, st * 128:(st + 1) * 128])
                state[st].clear()

            # Wavefront pipeline across supertiles: stage offsets
            # L1(t), L2(t-1), L3(t-2), t4(t-3), t5(t-4), ttt(t-5).
            # t5 one step after t4 so t5's matmuls never wait on the r4
            # evacuation within the same step (ditto ttt after t5).
            NSTG = 6
            emit_dma(0)
            emit_dma(1)
            for t in range(N_ST + NSTG - 1):
                if t + 2 < N_ST:
                    emit_dma(t + 2)

                def valid(off):
                    return 0 <= t - off < N_ST

                # piece order: dep-chained pieces (t6 -> t7 -> tails feed off
                # same-step ACT/Pool results) are spaced out between ready
                # streams so the PE's 4-deep dependency wait-queue never
                # head-of-line-blocks ready work
                pieces = []
                if valid(1):
                    pieces.append(lambda: emit_l2_chunk(t - 1, 0))
                if valid(0):
                    pieces.append(lambda: emit_l1_chunk(t, 0))
                if valid(3):
                    pieces.append(lambda: emit_t4(t - 3))
                if valid(2):
                    pieces.append(lambda: emit_l3_chunk(t - 2, 0))
                if valid(4):
                    pieces.append(lambda: emit_t5(t - 4))
                if valid(0):
                    pieces.append(lambda: emit_l1_chunk(t, 1))
                if valid(5):
                    pieces.append(lambda: emit_t6(t - 5))
                if valid(1):
                    pieces.append(lambda: emit_l2_chunk(t - 1, 1))
                if valid(5):
                    pieces.append(lambda: emit_t7(t - 5))
                if valid(2):
                    pieces.append(lambda: emit_l3_chunk(t - 2, 1))
                if valid(5):
                    pieces.append(lambda: emit_tails(t - 5))
                for p in pieces:
                    p()
    _split_multi_waits(nc)
    return nc


def kernel(**inputs):
    for l in range(1, 8):
        if np.abs(np.asarray(inputs[f"bt{l}"], np.float32)).max() > 0:
            return _numpy_forward(inputs)
        if np.asarray(inputs[f"g{l}"], np.float32).min() <= 0:
            return _numpy_forward(inputs)

    wpack, offs, wcols, ks = _build_consts(inputs)
    global _WCOLS
    _WCOLS = wcols

    x = np.asarray(inputs["x"], np.float32)
    xT = np.ascontiguousarray(x.T)               # [128, 524288]
    xhi = xT.astype(F16)
    b8 = float(np.asarray(inputs["b8"], np.float32).reshape(-1)[0])
    sq6_scale = float(np.sqrt(EPS) * ks[6])      # (scale*u6)^2 = eps*k7^2*u6^2

    nc = _build_program(offs, wpack.shape[1], sq6_scale)

    in_maps = []
    for c in range(N_CORES):
        s = slice(c * RPC, (c + 1) * RPC)
        in_maps.append({
            "xhi": np.ascontiguousarray(xhi[:, s]),
            "wpack": wpack,
        })

    from concourse.bass_utils import run_bass_kernel_spmd
    res = run_bass_kernel_spmd(nc, in_maps, core_ids=list(range(N_CORES)))

    # device ey layout [128, N_ST*128]: per supertile block [128,128]:
    # partitions 0:64 = E8, 64:128 = y; row-in-core = st*8192 + m*128 + j
    def unpack(arr):
        a = arr.reshape(128, N_ST, 128)
        e8 = a[0:64].transpose(1, 0, 2).reshape(-1)
        y = a[64:128].transpose(1, 0, 2).reshape(-1)
        return e8, y

    out = np.empty((ROWS, 1), np.float32)
    e8r = np.empty(ROWS, np.float64)
    for c in range(N_CORES):
        e8, y = unpack(res.results[c]["ey"].astype(np.float64))
        with np.errstate(invalid="ignore", divide="ignore"):
            out[c * RPC:(c + 1) * RPC, 0] = (y / np.sqrt(e8) + b8)
        e8r[c * RPC:(c + 1) * RPC] = e8

    finite = np.isfinite(e8r) & (e8r > 0)
    med = float(np.median(e8r[finite])) if finite.any() else 0.0
    if not np.isfinite(med) or med <= 0:
        return _numpy_forward(inputs).reshape(ROWS, 1)
    bad = (~finite) | (e8r < FLAG_RATIO * med) | (~np.isfinite(out[:, 0]))
    if bad.mean() > 0.30:
        return _numpy_forward(inputs).reshape(ROWS, 1)
    idx = np.nonzero(bad)[0]
    if idx.size:
        out[idx, 0] = _ref_rows(inputs, idx).astype(np.float32).ravel()
    return out
